# revision 40
# baseline (speedup 1.0000x reference)
"""BioZorro sparse-attention kernel for 8 Trainium2 NeuronCores.

Sharding: 8 cores = 2 batches x 4 token-quarters (384 own tokens each).
The zorro mask makes all non-fusion query rows fully masked -> uniform
softmax -> their attention output is mean(V); only the 16 fusion tokens
attend (over the 1536 non-fusion keys). Cross-core data per layer is two
small AllGathers: (A) V column sums (2KB) issued early, (B) fusion
flash-softmax partials (66KB), plus one tiny AllGather for pooling.

Compute layout: residual stream feature-major (tok^T [512, 400] f32).
All heavy matmuls run in fp8e4 DoubleRow (contract 256/instr, 2x rate):
activations are cast to paired [128,2,T] tiles; weights are host-packed
into one fp8 buffer per layer (single DMA, double-buffered). LayerNorms
are folded into consumers: raw-cast -> matmul immediately; the -mu
correction enters PSUM as a rank-1 matmul (host-precomputed column sums
x the device S row); rstd is applied at PSUM eviction (column-broadcast
or per-token scalars). Per-tensor power-of-2 fp8 scales are descaled via
free immediate-scale slots (exp/gelu/copy activations).
"""
import sys
sys.path.insert(0, "/opt/trn_rl_repo")
import numpy as np
import ml_dtypes

BF = ml_dtypes.bfloat16
F8 = ml_dtypes.float8_e4m3
OWN, FUS, TOK = 384, 16, 400
D, RIN, H, DH, IFF, DEPTH = 512, 1024, 8, 64, 1365, 4
NALL = 1552
B, NR, NA = 2, 768, 768
N_CORES = 8
IFFP = 1408           # x/gate block padding (11 x 128)
IFF2 = 1536           # FF2 contract padding (6 x 256)

# fp8 packed-weight segment offsets (cols in the per-layer [128, FCOLS])
SEG_WQ = 0            # [2kp][2sub][512]
SEG_WKV = 2048        # [2kp][2sub][1024]
SEG_WO = SEG_WKV + 4096   # [2kp][2sub][512]
SEG_W1 = SEG_WO + 2048    # [2kp][2sub][2*1408]
SEG_W2 = SEG_W1 + 11264   # [6jp][2sub][512]
FCOLS = SEG_W2 + 6144

_built = {}


def _pow2_scale(w, target=120.0):
    m = float(np.abs(w).max())
    if m <= 0:
        return 1.0
    return float(2.0 ** np.floor(np.log2(target / m)))


def _pack_pairs(w, scale):
    """[K, N] f64 -> [128, K//256, 2, N] fp8 DoubleRow lhsT layout."""
    K, N = w.shape
    assert K % 256 == 0
    out = (w * scale).astype(F8).reshape(K // 128, 128, N)
    # chunk k = rows 128k..128k+128; pair kp = (2kp, 2kp+1)
    out = out.transpose(1, 0, 2).reshape(128, K // 256, 2, N)
    return np.ascontiguousarray(out)


def build(num_devices=8, use_cc=True, scales=None):
    key = (num_devices, use_cc, scales)
    if key in _built:
        return _built[key]
    import concourse.tile as tile
    from concourse import bacc, mybir
    from concourse.masks import make_identity

    # Force Exp to resolve to natural_log_exp_and_others so Ln/Exp/Square
    # live in one ACT table set (Gelu still needs its own set; those two
    # swaps per layer are prefetched off the critical path with dummy ops).
    if not getattr(bacc, "_act_tables_patched", False):
        _orig_gat = bacc.get_activation_tables

        def _patched_gat(arch):
            tabs = _orig_gat(arch)
            exp_t = mybir.ActivationFunctionType.Exp
            for nm, fns in tabs.items():
                if nm != "natural_log_exp_and_others":
                    fns.discard(exp_t)
            return tabs

        bacc.get_activation_tables = _patched_gat
        bacc._act_tables_patched = True

    sq, skv, so, s1x, s1g, s2, se, spl, spo = scales
    f32 = mybir.dt.float32
    bf16 = mybir.dt.bfloat16
    f8 = mybir.dt.float8e4
    AF = mybir.ActivationFunctionType
    OP = mybir.AluOpType
    DR = mybir.MatmulPerfMode.DoubleRow

    nc = bacc.Bacc("TRN2", target_bir_lowering=False, debug=False,
                   enable_asserts=True, num_devices=num_devices)

    def din(name, shape, dt=f32):
        return nc.dram_tensor(name, shape, dt, kind="ExternalInput").ap()

    x8_t = din("x8", [128, 4, 2, OWN], f8)
    ew8_t = din("ew8", [128, 4, 2, D], f8)
    ecols_t = din("ecols", [128, 4, 3])
    erows_t = din("erows", [1, D + 2 * OWN], mybir.dt.bfloat16)
    fus_t = din("fus_t", [128, 4, FUS], f32)
    wpk_t = din("wpk", [DEPTH, 128, FCOLS], f8)
    # host rank-1 rows: per layer [wksum, wvsum, wqsum] each [512]
    rows_t = din("rows", [1, DEPTH * 3 * D], bf16)
    prow_t = din("prow", [1, 2 * D], bf16)    # pool [pwksum, pwvsum]
    pwkv8_t = din("pwkv8", [128, 2, 2, 2 * D], f8)
    pwo8_t = din("pwo8", [128, 2, 2, D], f8)
    pq2_t = din("pool_q2", [D, 1])
    out_u = nc.dram_tensor("out_u", [D, 1], f32, kind="ExternalOutput").ap()
    out_f = nc.dram_tensor("out_f", [1, D], f32, kind="ExternalOutput").ap()

    with tile.TileContext(nc) as tc:
        with tc.tile_pool(name="cst", bufs=1) as cst, \
             tc.tile_pool(name="wp", bufs=2) as wp, \
             tc.tile_pool(name="ac", bufs=2) as ac, \
             tc.tile_pool(name="pgen", bufs=4, space="PSUM") as pgen, \
             tc.tile_pool(name="pacc", bufs=4, space="PSUM") as pacc, \
             tc.tile_pool(name="dramp", bufs=2, space="DRAM") as dramp:

            ident = cst.tile([128, 128], bf16, name="ident")
            make_identity(nc, ident[:])
            ones128 = cst.tile([128, 1], bf16, name="ones128")
            nc.vector.memset(ones128[:], 1.0)
            ones1 = cst.tile([1, 128], bf16, name="ones1")
            nc.vector.memset(ones1[:], 1.0)
            epsc = cst.tile([128, 1], f32, name="epsc")
            nc.vector.memset(epsc[:], 1e-5)
            oi512 = cst.tile([128, 1], bf16, name="oi512")
            nc.vector.memset(oi512[:], 1.0 / 512)
            ones8p = cst.tile([128, 1], f8, name="ones8p")
            nc.vector.memset(ones8p[:], 0.125)
            cinvf = 1.0 / (s1x * s2)

            def load_cols(dram_ap, n, tag, rows=128):
                ts = []
                for c in range(n):
                    t = wp.tile([rows, 1], f32, tag=f"{tag}{c}", bufs=1,
                                name=f"{tag}{c}")
                    nc.sync.dma_start(out=t[:],
                                      in_=dram_ap[rows * c:rows * (c + 1), :])
                    ts.append(t)
                return ts

            # ---------- one-time loads (embed inputs first; w0 after) ----------
            x8 = ac.tile([128, 4, 2, OWN], f8, tag="x8", bufs=1, name="x8")
            nc.sync.dma_start(out=x8[:], in_=x8_t)
            ew8 = wp.tile([128, 4, 2, D], f8, tag="ew8", bufs=1, name="ew8")
            nc.sync.dma_start(out=ew8[:], in_=ew8_t)
            erows = wp.tile([1, D + 2 * OWN], bf16, tag="erows", bufs=1,
                            name="erows")
            nc.sync.dma_start(out=erows[:], in_=erows_t)
            ecols = wp.tile([128, 4, 3], f32, tag="ecols", bufs=1,
                            name="ecols")
            nc.sync.dma_start(out=ecols[:], in_=ecols_t)
            rows = wp.tile([1, DEPTH * 3 * D], bf16, tag="rows", bufs=1,
                           name="rows")
            nc.sync.dma_start(out=rows[:], in_=rows_t)
            w0 = wp.tile([128, FCOLS], f8, tag="wpk", bufs=2, name="wpk0")
            nc.sync.dma_start(out=w0[:], in_=wpk_t[0])
            erow = erows[:, 0:D]
            Seb = erows[:, D:D + OWN]
            rstdeb = erows[:, D + OWN:D + 2 * OWN]
            ebs = [ecols[:, c, 0:1] for c in range(4)]
            eg2s = [ecols[:, c, 1:2] for c in range(4)]
            eb2s = [ecols[:, c, 2:3] for c in range(4)]

            # dummy exp to preload the nlexp ACT table during initial DMAs
            dtab = ac.tile([1, 1], f32, tag="dtab", bufs=2, name="dtab")
            nc.scalar.activation(out=dtab[:], in_=epsc[0:1, :], func=AF.Exp)

            # ---------- embed (input-LN stats host-precomputed) ----------
            bRe_p = pgen.tile([128, OWN], f32, tag="g", name="bRe")
            nc.tensor.matmul(bRe_p[:], ones1[:], rstdeb, start=True,
                             stop=True)
            bRe = ac.tile([128, OWN], bf16, tag="bR", bufs=2, name="bReb")
            nc.vector.tensor_copy(out=bRe[:], in_=bRe_p[:])

            hb = []
            for mc in range(4):
                ps = pgen.tile([128, OWN], f32, tag="g", name=f"embp{mc}")
                for kp in range(4):
                    nc.tensor.matmul(ps[:], ew8[:, kp, :, 128 * mc:128 * (mc + 1)],
                                     x8[:, kp, :, :], start=(kp == 0),
                                     stop=False, perf_mode=DR)
                nc.tensor.matmul(ps[:], erow[:, 128 * mc:128 * (mc + 1)],
                                 Seb, start=False, stop=True)
                t1 = ac.tile([128, OWN], bf16, tag="embt", bufs=2,
                             name=f"embt{mc}")
                nc.vector.tensor_mul(out=t1[:], in0=ps[:], in1=bRe[:])
                t2 = ac.tile([128, OWN], bf16, tag=f"hb{mc}", bufs=1,
                             name=f"hb{mc}")
                nc.vector.tensor_scalar_add(out=t2[:], in0=t1[:],
                                            scalar1=ebs[mc])
                hb.append(t2)

            # embed LN2 (explicit normalize into f32 tok)
            S2e = pgen.tile([1, OWN], f32, tag="g", name="S2e")
            for c in range(4):
                nc.tensor.matmul(S2e[:], oi512[:], hb[c][:],
                                 start=(c == 0), stop=(c == 3))
            x2e = []
            for c in range(4):
                t = ac.tile([128, OWN], bf16, tag="xsq", bufs=4,
                            name=f"x2e{c}")
                if c % 2 == 0:
                    nc.vector.tensor_mul(out=t[:], in0=hb[c][:], in1=hb[c][:])
                else:
                    nc.scalar.activation(out=t[:], in_=hb[c][:],
                                         func=AF.Square)
                x2e.append(t)
            Q2e = pgen.tile([1, OWN], f32, tag="g", name="Q2e")
            for c in range(4):
                nc.tensor.matmul(Q2e[:], oi512[:], x2e[c][:],
                                 start=(c == 0), stop=(c == 3))
            m22 = ac.tile([1, OWN], f32, tag="rowf", bufs=6, name="m22")
            nc.scalar.activation(out=m22[:], in_=S2e[:], func=AF.Square)
            var2 = ac.tile([1, OWN], f32, tag="rowf", bufs=6, name="var2e")
            nc.vector.tensor_sub(out=var2[:], in0=Q2e[:], in1=m22[:])
            rstd2e = ac.tile([1, OWN], f32, tag="rowf", bufs=6, name="rstd2e")
            nc.scalar.activation(out=rstd2e[:], in_=var2[:], func=AF.Ln,
                                 bias=epsc[0:1, :])
            nc.scalar.activation(out=rstd2e[:], in_=rstd2e[:], func=AF.Exp,
                                 scale=-0.5)
            pr2 = ac.tile([1, 2 * OWN], bf16, tag="rowb2", bufs=2, name="pr2e")
            nc.vector.tensor_copy(out=pr2[:, 0:OWN], in_=rstd2e[:])
            mre = ac.tile([1, OWN], f32, tag="rowf", bufs=6, name="mre")
            nc.vector.tensor_mul(out=mre[:], in0=S2e[:], in1=rstd2e[:])
            nc.vector.tensor_copy(out=pr2[:, OWN:2 * OWN], in_=mre[:])
            bR2e_p = pgen.tile([128, OWN], f32, tag="g", name="bR2e")
            nc.tensor.matmul(bR2e_p[:], ones1[:], pr2[:, 0:OWN], start=True,
                             stop=True)
            bR2e = ac.tile([128, OWN], bf16, tag="bR", bufs=2, name="bR2eb")
            nc.vector.tensor_copy(out=bR2e[:], in_=bR2e_p[:])
            bM2e_p = pgen.tile([128, OWN], f32, tag="g", name="bM2e")
            nc.tensor.matmul(bM2e_p[:], ones1[:], pr2[:, OWN:2 * OWN],
                             start=True, stop=True)
            bM2e = ac.tile([128, OWN], bf16, tag="bM", bufs=2, name="bM2eb")
            nc.vector.tensor_copy(out=bM2e[:], in_=bM2e_p[:])

            tok = [ac.tile([128, TOK], f32, tag=f"tok{c}", bufs=1,
                           name=f"tok{c}") for c in range(4)]
            for c in range(4):
                t1 = ac.tile([128, OWN], bf16, tag="embt", bufs=2,
                             name=f"eln{c}")
                nc.vector.tensor_mul(out=t1[:], in0=hb[c][:], in1=bR2e[:])
                nc.vector.tensor_sub(out=t1[:], in0=t1[:], in1=bM2e[:])
                nc.vector.tensor_scalar(out=tok[c][:, 0:OWN], in0=t1[:],
                                        scalar1=eg2s[c], scalar2=eb2s[c],
                                        op0=OP.mult, op1=OP.add)
                nc.sync.dma_start(out=tok[c][:, OWN:TOK], in_=fus_t[:, c, :])

            tok_chunks = [(0, 128), (128, 256), (256, 384), (384, 400)]
            rg = [[0, 1, 2, 3], [4, 5, 6, 7]]
            psO_prev = None

            # rank-1 row views per layer
            def lrow(l, which):
                base = (l * 3 + which) * D
                return rows[:, base:base + D]

            # ---------- layers ----------
            for l in range(DEPTH):
                wT = w0 if l == 0 else wp.tile([128, FCOLS], f8, tag="wpk",
                                               bufs=2, name=f"wpk{l}")
                if l > 0:
                    nc.sync.dma_start(out=wT[:], in_=wpk_t[l])
                wq8 = wT[:, SEG_WQ:SEG_WKV].rearrange(
                    "p (kp s o) -> p kp s o", kp=2, s=2)
                wkv8 = wT[:, SEG_WKV:SEG_WO].rearrange(
                    "p (kp s o) -> p kp s o", kp=2, s=2)
                wo8 = wT[:, SEG_WO:SEG_W1].rearrange(
                    "p (kp s o) -> p kp s o", kp=2, s=2)
                w18 = wT[:, SEG_W1:SEG_W2].rearrange(
                    "p (kp s o) -> p kp s o", kp=2, s=2)
                w28 = wT[:, SEG_W2:FCOLS].rearrange(
                    "p (jp s o) -> p jp s o", jp=6, s=2)

                # --- LN1 prep: residual + fp8 raw cast ---
                tr8 = [ac.tile([128, 2, TOK], f8, tag=f"tr8{kp}", bufs=1,
                               name=f"tr8_{kp}") for kp in range(2)]
                for c in range(4):
                    if psO_prev is not None:
                        t = ac.tile([128, TOK], bf16, tag="psot", bufs=2,
                                    name=f"psot{c}")
                        nc.scalar.activation(out=t[:], in_=psO_prev[c][:],
                                             func=AF.Copy, scale=cinvf)
                        nc.vector.tensor_add(out=tok[c][:], in0=tok[c][:],
                                             in1=t[:])
                    nc.vector.tensor_copy(out=tr8[c // 2][:, c % 2, :],
                                          in_=tok[c][:])
                psO_prev = None

                # --- stats1 (rank-1 style folded LN) ---
                S1 = pgen.tile([1, TOK], f32, tag="g", name="S1")
                for i in range(4):
                    nc.tensor.matmul(S1[:], ones8p[:], tr8[i // 2][:, i % 2, :],
                                     start=(i == 0), stop=(i == 3))
                S1b = ac.tile([1, TOK], bf16, tag="rowb", bufs=4, name="S1b")
                nc.scalar.activation(out=S1b[:], in_=S1[:], func=AF.Copy)
                m2 = ac.tile([1, TOK], f32, tag="rowf", bufs=6, name="m2")
                nc.scalar.activation(out=m2[:], in_=S1[:], func=AF.Square,
                                     scale=1.0 / 64)
                xsq = []
                for kp in range(2):
                    t = ac.tile([128, 2, TOK], bf16, tag="xsq", bufs=4,
                                name=f"xsq{kp}")
                    if kp == 0:
                        nc.vector.tensor_mul(out=t[:], in0=tr8[kp][:],
                                             in1=tr8[kp][:])
                    else:
                        nc.scalar.activation(out=t[:], in_=tr8[kp][:],
                                             func=AF.Square)
                    xsq.append(t)

                # --- V raw (+ early evict, rstd folded into consumers) ---
                V = []
                for i, (a, b) in enumerate(tok_chunks):
                    m = b - a
                    ps = pacc.tile([128, D], f32, tag="a", name=f"v{i}")
                    for kp in range(2):
                        nc.tensor.matmul(ps[0:m, :], tr8[kp][:, :, a:b],
                                         wkv8[:, kp, :, D:2 * D],
                                         start=(kp == 0), stop=False,
                                         perf_mode=DR)
                    nc.tensor.matmul(ps[0:m, :], S1b[:, a:b],
                                     lrow(l, 1), start=False, stop=True)
                    sV = ac.tile([128, D], bf16, tag=f"V{i}", bufs=1,
                                 name=f"Vb{i}")
                    nc.scalar.activation(out=sV[0:m, :], in_=ps[0:m, :],
                                         func=AF.Copy, scale=1.0 / skv)
                    V.append(sV)

                # --- K^T, Qf raw matmuls ---
                ktp, qfp = [], []
                for mc in range(4):
                    ps = pacc.tile([128, OWN], f32, tag="a", name=f"kt{mc}")
                    for kp in range(2):
                        nc.tensor.matmul(ps[:],
                                         wkv8[:, kp, :, 128 * mc:128 * (mc + 1)],
                                         tr8[kp][:, :, 0:OWN],
                                         start=(kp == 0), stop=False,
                                         perf_mode=DR)
                    nc.tensor.matmul(ps[:], lrow(l, 0)[:, 128 * mc:128 * (mc + 1)],
                                     S1b[:, 0:OWN], start=False, stop=True)
                    ktp.append(ps)
                # --- stats1 tail ---
                Q1 = pgen.tile([1, TOK], f32, tag="g", name="Q1")
                for i in range(4):
                    nc.tensor.matmul(Q1[:], oi512[:], xsq[i // 2][:, i % 2, :],
                                     start=(i == 0), stop=(i == 3))
                var = ac.tile([1, TOK], f32, tag="rowf", bufs=6, name="var")
                nc.vector.tensor_sub(out=var[:], in0=Q1[:], in1=m2[:])
                rstd = ac.tile([1, TOK], f32, tag="rowf", bufs=6, name="rstd")
                nc.scalar.activation(out=rstd[:], in_=var[:], func=AF.Ln,
                                     bias=epsc[0:1, :])
                nc.scalar.activation(out=rstd[:], in_=rstd[:], func=AF.Exp,
                                     scale=-0.5)
                rstdb = ac.tile([1, TOK], bf16, tag="rowb", bufs=4,
                                name="rstdb")
                nc.vector.tensor_copy(out=rstdb[:], in_=rstd[:])
                bR1_p = pgen.tile([128, TOK], f32, tag="g", name="bR1")
                nc.tensor.matmul(bR1_p[:], ones1[:], rstdb[:], start=True,
                                 stop=True)
                bR1 = ac.tile([128, TOK], bf16, tag="bR", bufs=2, name="bR1b")
                nc.vector.tensor_copy(out=bR1[:], in_=bR1_p[:])
                rT_p = pgen.tile([128, 4], f32, tag="g", name="rTp")
                for c, (a_, b_) in enumerate(tok_chunks):
                    nc.tensor.matmul(rT_p[0:b_ - a_, c:c + 1],
                                     rstdb[:, a_:b_],
                                     ones1[0:1, 0:1], start=True, stop=True)
                rT = ac.tile([128, 4], bf16, tag="rT", bufs=2, name="rT")
                nc.vector.tensor_copy(out=rT[:], in_=rT_p[:])
                rTs = ac.tile([128, 4], f32, tag="rTs", bufs=2, name="rTs")
                nc.vector.tensor_copy(out=rTs[:], in_=rT_p[:])

                # --- Qf raw (after stats-tail matmuls; "a" slots free
                # once kt evictions land) ---
                for mc in range(4):
                    ps = pacc.tile([128, FUS], f32, tag="a", name=f"qf{mc}")
                    for kp in range(2):
                        nc.tensor.matmul(ps[:],
                                         wq8[:, kp, :, 128 * mc:128 * (mc + 1)],
                                         tr8[kp][:, :, OWN:TOK],
                                         start=(kp == 0), stop=False,
                                         perf_mode=DR)
                    nc.tensor.matmul(ps[:], lrow(l, 2)[:, 128 * mc:128 * (mc + 1)],
                                     S1b[:, OWN:TOK], start=False, stop=True)
                    qfp.append(ps)

                # --- vsums (rstd via rhs) -> exchange A ---
                PA = ac.tile([128, 4], f32, tag="PA", bufs=2, name="PA")
                for c in range(4):
                    ps = pgen.tile([128, 1], f32, tag="g", name=f"vs{c}")
                    for j in range(3):
                        nc.tensor.matmul(ps[:], V[j][:, 128 * c:128 * (c + 1)],
                                         rT[:, j:j + 1], start=(j == 0),
                                         stop=(j == 2))
                    nc.vector.tensor_copy(out=PA[:, c:c + 1], in_=ps[:])
                vfu = ac.tile([128, 4], f32, tag="vfu", bufs=2, name="vfu")
                for c in range(4):
                    ps = pgen.tile([128, 1], f32, tag="g", name=f"vf{c}")
                    nc.tensor.matmul(ps[:], V[3][0:FUS, 128 * c:128 * (c + 1)],
                                     rT[0:FUS, 3:4], start=True, stop=True)
                    nc.vector.tensor_copy(out=vfu[:, c:c + 1], in_=ps[:])
                pinA = dramp.tile([128, 4], f32, tag="pinA", bufs=2,
                                  name="pinA")
                nc.sync.dma_start(out=pinA[:], in_=PA[:])
                RA = ac.tile([128, 4, 4], f32, tag="RA", bufs=2, name="RA")
                if use_cc:
                    poutA = dramp.tile([4 * 128, 4], f32, tag="poutA", bufs=2,
                                       name="poutA")
                    nc.gpsimd.collective_compute(
                        "AllGather", OP.bypass, replica_groups=rg,
                        ins=[pinA.opt()], outs=[poutA.opt()])
                    nc.sync.dma_start(
                        out=RA[:], in_=poutA.rearrange("(r p) f -> p r f", r=4))
                else:
                    nc.sync.dma_start(
                        out=RA[:],
                        in_=pinA.rearrange("(r p) f -> p r f", r=1)
                        .to_broadcast((128, 4, 4)))

                # --- kt/qf evictions (x rstd columns) ---
                kt = []
                for mc in range(4):
                    sK = ac.tile([128, OWN], bf16, tag=f"kt{mc}", bufs=1,
                                 name=f"ktb{mc}")
                    nc.vector.tensor_mul(out=sK[:], in0=ktp[mc][:],
                                         in1=bR1[:, 0:OWN])
                    kt.append(sK)
                qf = []
                for mc in range(4):
                    sQ = ac.tile([128, 32], bf16, tag=f"qf{mc}", bufs=1,
                                 name=f"qfb{mc}")
                    nc.vector.memset(sQ[:, FUS:32], 0.0)
                    nc.vector.tensor_mul(out=sQ[:, 0:FUS], in0=qfp[mc][:],
                                         in1=bR1[:, OWN:TOK])
                    qf.append(sQ)

                # --- scores + exp ---
                E, lacc = [], []
                for t in range(2):
                    sp = pgen.tile([128, OWN], f32, tag="g", name=f"sp{t}")
                    for i in range(4):
                        h = 4 * t + i
                        ch, base = h // 2, (h % 2) * 64
                        nc.tensor.matmul(sp[32 * i:32 * i + 32, :],
                                         qf[ch][base:base + 64, 0:32],
                                         kt[ch][base:base + 64, :],
                                         start=True, stop=True,
                                         tile_position=(base, 32 * i))
                    e = ac.tile([128, OWN], bf16, tag=f"e{t}", bufs=1,
                                name=f"e{t}")
                    la = ac.tile([128, 1], f32, tag=f"la{t}", bufs=2,
                                 name=f"la{t}")
                    nc.scalar.activation(out=e[:], in_=sp[:], func=AF.Exp,
                                         scale=1.0 / (sq * skv),
                                         accum_out=la[:])
                    E.append(e)
                    lacc.append(la)

                # --- E^T + ACC -> exchange B ---
                ET = [[None] * 3 for _ in range(2)]
                for t in range(2):
                    for j in range(3):
                        pt = pgen.tile([128, 128], bf16, tag="g",
                                       name=f"et{t}{j}")
                        nc.tensor.transpose(pt[:], E[t][:, 128 * j:128 * (j + 1)],
                                            ident[:])
                        s = ac.tile([128, 128], bf16, tag=f"ET{t}{j}", bufs=1,
                                    name=f"ETb{t}{j}")
                        nc.vector.tensor_scalar_mul(out=s[:], in0=pt[:],
                                                    scalar1=rTs[:, j:j + 1])
                        ET[t][j] = s
                PB = ac.tile([128, 130], f32, tag="PB", bufs=2, name="PB")
                nc.vector.tensor_copy(out=PB[:, 0:1], in_=lacc[0][:])
                nc.vector.tensor_copy(out=PB[:, 1:2], in_=lacc[1][:])
                for t in range(2):
                    acc = pacc.tile([128, 64], f32, tag="a", name=f"acc{t}")
                    for i in range(4):
                        h = 4 * t + i
                        for j in range(3):
                            nc.tensor.matmul(acc[32 * i:32 * i + 32, :],
                                             ET[t][j][:, 32 * i:32 * i + 32],
                                             V[j][:, DH * h:DH * (h + 1)],
                                             start=(j == 0), stop=(j == 2),
                                             tile_position=(0, 32 * i))
                    nc.vector.tensor_copy(out=PB[:, 2 + 64 * t:66 + 64 * t],
                                          in_=acc[:])
                pinB = dramp.tile([128, 130], f32, tag="pinB", bufs=2,
                                  name="pinB")
                nc.sync.dma_start(out=pinB[:], in_=PB[:])
                RB = ac.tile([128, 4, 130], f32, tag="RB", bufs=2, name="RB")
                if use_cc:
                    poutB = dramp.tile([4 * 128, 130], f32, tag="poutB",
                                       bufs=2, name="poutB")
                    nc.gpsimd.collective_compute(
                        "AllGather", OP.bypass, replica_groups=rg,
                        ins=[pinB.opt()], outs=[poutB.opt()])
                    nc.sync.dma_start(
                        out=RB[:], in_=poutB.rearrange("(r p) f -> p r f", r=4))
                else:
                    nc.sync.dma_start(
                        out=RB[:],
                        in_=pinB.rearrange("(r p) f -> p r f", r=1)
                        .to_broadcast((128, 4, 130)))

                # --- A-combine -> uniform delta -> own-col LN2 prep ---
                vsb = ac.tile([128, 4], f32, tag="vsb", bufs=2, name="vsb")
                nc.vector.tensor_reduce(out=vsb[:],
                                        in_=RA[:].rearrange("p r f -> p f r"),
                                        axis=mybir.AxisListType.X,
                                        op=OP.add)
                nc.vector.tensor_add(out=vsb[:], in0=vsb[:], in1=vfu[:])
                vsb8 = ac.tile([128, 2, 2], f8, tag="vsb8", bufs=2,
                               name="vsb8")
                nc.vector.tensor_copy(
                    out=vsb8[:],
                    in_=vsb[:].rearrange("p (kp s) -> p s kp", kp=2))
                dup = pgen.tile([128, 4], f32, tag="g", name="dup")
                for c in range(4):
                    for kp in range(2):
                        nc.tensor.matmul(dup[:, c:c + 1],
                                         wo8[:, kp, :, 128 * c:128 * (c + 1)],
                                         vsb8[:, :, kp:kp + 1],
                                         start=(kp == 0), stop=(kp == 1),
                                         perf_mode=DR)
                dus = ac.tile([128, 4], f32, tag="dus", bufs=2, name="dus")
                nc.scalar.activation(out=dus[:], in_=dup[:], func=AF.Copy,
                                     scale=1.0 / (so * NALL))
                t28 = [ac.tile([128, 2, TOK], f8, tag=f"t28_{kp}", bufs=1,
                               name=f"t28_{kp}") for kp in range(2)]
                for c in range(4):
                    nc.vector.tensor_scalar_add(out=tok[c][:, 0:OWN],
                                                in0=tok[c][:, 0:OWN],
                                                scalar1=dus[:, c:c + 1])
                    nc.vector.tensor_copy(out=t28[c // 2][:, c % 2, 0:OWN],
                                          in_=tok[c][:, 0:OWN])
                S2 = pgen.tile([1, OWN], f32, tag="g", name="S2")
                for i in range(4):
                    nc.tensor.matmul(S2[:], ones8p[:],
                                     t28[i // 2][:, i % 2, 0:OWN],
                                     start=(i == 0), stop=(i == 3))
                mur2b = ac.tile([1, TOK], bf16, tag="rowb", bufs=4,
                                name="mur2b")
                nc.scalar.activation(out=mur2b[:, 0:OWN], in_=S2[:],
                                     func=AF.Copy, scale=1.0 / 64)
                m2_2 = ac.tile([1, TOK], f32, tag="rowf", bufs=6, name="m2_2")
                nc.scalar.activation(out=m2_2[:, 0:OWN], in_=S2[:],
                                     func=AF.Square, scale=1.0 / 64)
                bMu_p = pgen.tile([128, OWN], f32, tag="g", name="bMu")
                nc.tensor.matmul(bMu_p[:], ones1[:], mur2b[:, 0:OWN],
                                 start=True, stop=True)
                bMu = ac.tile([128, TOK], bf16, tag="bM", bufs=2, name="bMub")
                nc.vector.tensor_copy(out=bMu[:, 0:OWN], in_=bMu_p[:])
                for c in range(4):
                    nc.vector.tensor_sub(out=tok[c][:, 0:OWN],
                                         in0=tok[c][:, 0:OWN],
                                         in1=bMu[:, 0:OWN])
                xsq2 = []
                for kp in range(2):
                    t = ac.tile([128, 2, TOK], bf16, tag="xsq", bufs=4,
                                name=f"xsq2{kp}")
                    nc.scalar.activation(out=t[:, :, 0:OWN],
                                         in_=t28[kp][:, :, 0:OWN],
                                         func=AF.Square)
                    xsq2.append(t)
                Q2 = pgen.tile([1, OWN], f32, tag="g", name="Q2")
                for i in range(4):
                    nc.tensor.matmul(Q2[:], oi512[:],
                                     xsq2[i // 2][:, i % 2, 0:OWN],
                                     start=(i == 0), stop=(i == 3))

                # --- own-col stats2 tail (overlaps exchange B) ---
                var2 = ac.tile([1, TOK], f32, tag="rowf", bufs=6, name="var2")
                nc.vector.tensor_sub(out=var2[:, 0:OWN], in0=Q2[:],
                                     in1=m2_2[:, 0:OWN])
                rstd2 = ac.tile([1, TOK], f32, tag="rowf", bufs=6,
                                name="rstd2")
                nc.scalar.activation(out=rstd2[:, 0:OWN], in_=var2[:, 0:OWN],
                                     func=AF.Ln, bias=epsc[0:1, :])
                nc.scalar.activation(out=rstd2[:, 0:OWN], in_=rstd2[:, 0:OWN],
                                     func=AF.Exp, scale=-0.5)
                rstd2b = ac.tile([1, TOK], bf16, tag="rowb", bufs=4,
                                 name="rstd2b")
                nc.vector.tensor_copy(out=rstd2b[:, 0:OWN],
                                      in_=rstd2[:, 0:OWN])
                bR2_p = pgen.tile([128, OWN], f32, tag="g", name="bR2")
                nc.tensor.matmul(bR2_p[:], ones1[:], rstd2b[:, 0:OWN],
                                 start=True, stop=True)
                bR2 = ac.tile([128, TOK], bf16, tag="bR", bufs=2, name="bR2b")
                nc.vector.tensor_copy(out=bR2[:, 0:OWN], in_=bR2_p[:])
                xc8 = [ac.tile([128, 2, TOK], f8, tag=f"xc8_{kp}", bufs=1,
                               name=f"xc8_{kp}") for kp in range(2)]
                for c in range(4):
                    nc.vector.tensor_mul(out=xc8[c // 2][:, c % 2, 0:OWN],
                                         in0=tok[c][:, 0:OWN],
                                         in1=bR2[:, 0:OWN])

                # --- B-combine -> fusion delta -> fusion-col LN2 prep ---
                PT = ac.tile([128, 130], f32, tag="cmbB", bufs=3, name="PT")
                nc.vector.tensor_reduce(out=PT[:],
                                        in_=RB[:].rearrange("p r f -> p f r"),
                                        axis=mybir.AxisListType.X,
                                        op=OP.add)
                linv = ac.tile([128, 2], f32, tag="linv", bufs=2, name="linv")
                nc.vector.reciprocal(out=linv[:], in_=PT[:, 0:2])
                of = []
                for t in range(2):
                    s = ac.tile([128, 64], bf16, tag=f"of{t}", bufs=1,
                                name=f"of{t}")
                    nc.vector.tensor_scalar_mul(out=s[:],
                                                in0=PT[:, 2 + 64 * t:66 + 64 * t],
                                                scalar1=linv[:, t:t + 1])
                    of.append(s)
                ofI8 = [ac.tile([128, 2, FUS], f8, tag=f"ofI{kp}", bufs=1,
                                name=f"ofI{kp}") for kp in range(2)]
                for kc in range(4):
                    pt = pacc.tile([128, 32], bf16, tag="a", name=f"ofIp{kc}")
                    for half in range(2):
                        h = 2 * kc + half
                        t, i = h // 4, h % 4
                        nc.tensor.transpose(
                            pt[64 * half:64 * half + 64, :],
                            of[t][32 * i:32 * i + 32, :],
                            ident[32 * i:32 * i + 32, 32 * i:32 * i + 32],
                            tile_position=(32 * i, 64 * half))
                    nc.vector.tensor_copy(out=ofI8[kc // 2][:, kc % 2, :],
                                          in_=pt[:, 0:FUS])
                for c in range(4):
                    dfp = pacc.tile([128, FUS], f32, tag="a", name=f"dfp{c}")
                    for kp in range(2):
                        nc.tensor.matmul(dfp[:],
                                         wo8[:, kp, :, 128 * c:128 * (c + 1)],
                                         ofI8[kp][:],
                                         start=(kp == 0), stop=(kp == 1),
                                         perf_mode=DR)
                    dft = ac.tile([128, FUS], bf16, tag="dft", bufs=2,
                                  name=f"dftb{c}")
                    nc.scalar.activation(out=dft[:], in_=dfp[:],
                                         func=AF.Copy, scale=1.0 / so)
                    nc.vector.tensor_add(out=tok[c][:, OWN:TOK],
                                         in0=tok[c][:, OWN:TOK], in1=dft[:])
                    nc.vector.tensor_copy(out=t28[c // 2][:, c % 2, OWN:TOK],
                                          in_=tok[c][:, OWN:TOK])
                S2f = pgen.tile([1, FUS], f32, tag="g", name="S2f")
                for i in range(4):
                    nc.tensor.matmul(S2f[:], ones8p[:],
                                     t28[i // 2][:, i % 2, OWN:TOK],
                                     start=(i == 0), stop=(i == 3))
                for kp in range(2):
                    nc.scalar.activation(out=xsq2[kp][:, :, OWN:TOK],
                                         in_=t28[kp][:, :, OWN:TOK],
                                         func=AF.Square)
                Q2f = pgen.tile([1, FUS], f32, tag="g", name="Q2f")
                for i in range(4):
                    nc.tensor.matmul(Q2f[:], oi512[:],
                                     xsq2[i // 2][:, i % 2, OWN:TOK],
                                     start=(i == 0), stop=(i == 3))

                # --- fusion-col centering + tail (post exchange B) ---
                nc.scalar.activation(out=mur2b[:, OWN:TOK], in_=S2f[:],
                                     func=AF.Copy, scale=1.0 / 64)
                bMuf_p = pgen.tile([128, FUS], f32, tag="g", name="bMuf")
                nc.tensor.matmul(bMuf_p[:], ones1[:],
                                 mur2b[:, OWN:TOK], start=True, stop=True)
                nc.vector.tensor_copy(out=bMu[:, OWN:TOK],
                                      in_=bMuf_p[:])
                for c in range(4):
                    nc.vector.tensor_sub(out=tok[c][:, OWN:TOK],
                                         in0=tok[c][:, OWN:TOK],
                                         in1=bMu[:, OWN:TOK])
                nc.scalar.activation(out=m2_2[:, OWN:TOK], in_=S2f[:],
                                     func=AF.Square, scale=1.0 / 64)
                nc.vector.tensor_sub(out=var2[:, OWN:TOK], in0=Q2f[:],
                                     in1=m2_2[:, OWN:TOK])
                nc.scalar.activation(out=rstd2[:, OWN:TOK],
                                     in_=var2[:, OWN:TOK],
                                     func=AF.Ln, bias=epsc[0:1, :])
                nc.scalar.activation(out=rstd2[:, OWN:TOK],
                                     in_=rstd2[:, OWN:TOK],
                                     func=AF.Exp, scale=-0.5)
                nc.vector.tensor_copy(out=rstd2b[:, OWN:TOK],
                                      in_=rstd2[:, OWN:TOK])
                bR2f_p = pgen.tile([128, FUS], f32, tag="g", name="bR2f")
                nc.tensor.matmul(bR2f_p[:], ones1[:],
                                 rstd2b[:, OWN:TOK], start=True, stop=True)
                nc.vector.tensor_copy(out=bR2[:, OWN:TOK],
                                      in_=bR2f_p[:])
                # dummy gelu: preload the gelu table while FF1 matmuls run
                dt2 = ac.tile([1, 1], f32, tag="dtab", bufs=2, name="dt2")
                nc.scalar.activation(out=dt2[:], in_=rstd2b[0:1, 0:1],
                                     func=AF.Gelu)
                for c in range(4):
                    nc.vector.tensor_mul(out=xc8[c // 2][:, c % 2, OWN:TOK],
                                         in0=tok[c][:, OWN:TOK],
                                         in1=bR2[:, OWN:TOK])

                # --- FF1 / GEGLU / FF2 ---
                gt8 = [ac.tile([128, 2, TOK], f8, tag=f"gt{jp}", bufs=1,
                               name=f"gt{jp}") for jp in range(6)]
                nc.vector.memset(gt8[5][:, 1, :], 0.0)
                for j in range(11):
                    px = pgen.tile([128, TOK], f32, tag="g", name=f"fx{j}")
                    pg = pacc.tile([128, TOK], f32, tag="a", name=f"fg{j}")
                    for kp in range(2):
                        nc.tensor.matmul(px[:], w18[:, kp, :, 128 * j:128 * (j + 1)],
                                         xc8[kp][:], start=(kp == 0),
                                         stop=(kp == 1), perf_mode=DR)
                    for kp in range(2):
                        nc.tensor.matmul(
                            pg[:],
                            w18[:, kp, :, IFFP + 128 * j:IFFP + 128 * (j + 1)],
                            xc8[kp][:], start=(kp == 0),
                            stop=(kp == 1), perf_mode=DR)
                    gg = ac.tile([128, TOK], bf16, tag="gg", bufs=3,
                                 name=f"gg{j}")
                    nc.scalar.activation(out=gg[:], in_=pg[:], func=AF.Gelu,
                                         scale=1.0 / s1g)
                    nc.vector.tensor_mul(out=gt8[j // 2][:, j % 2, :],
                                         in0=gg[:], in1=px[:])
                # dummy exp: preload nlexp for the next layer during FF2
                dt3 = ac.tile([1, 1], f32, tag="dtab", bufs=2, name="dt3")
                nc.scalar.activation(out=dt3[:], in_=gt8[5][0:1, 0, 0:1],
                                     func=AF.Exp)
                psO_prev = []
                for c in range(4):
                    psO = pacc.tile([128, TOK], f32, tag="a", name=f"fo{c}")
                    for jp in range(6):
                        nc.tensor.matmul(psO[:], w28[:, jp, :, 128 * c:128 * (c + 1)],
                                         gt8[jp][:], start=(jp == 0),
                                         stop=(jp == 5), perf_mode=DR)
                    psO_prev.append(psO)

            # ---------- pool ----------
            pwkv8 = wp.tile([128, 2, 2, 2 * D], f8, tag="pwkv8", bufs=1,
                            name="pwkv8")
            nc.sync.dma_start(out=pwkv8[:], in_=pwkv8_t)
            pwo8 = wp.tile([128, 2, 2, D], f8, tag="pwo8", bufs=1,
                           name="pwo8")
            nc.sync.dma_start(out=pwo8[:], in_=pwo8_t)
            prow = wp.tile([1, 2 * D], bf16, tag="prow", bufs=1, name="prow")
            nc.sync.dma_start(out=prow[:], in_=prow_t)
            pq2s = load_cols(pq2_t, 4, "pq2")

            # final LN (rank-1 folded) over tok + last FF residual
            tf8 = [ac.tile([128, 2, TOK], f8, tag=f"tr8{kp}", bufs=1,
                           name=f"tf8_{kp}") for kp in range(2)]
            for c in range(4):
                t = ac.tile([128, TOK], bf16, tag="psot", bufs=2,
                            name=f"fpsot{c}")
                nc.scalar.activation(out=t[:], in_=psO_prev[c][:],
                                     func=AF.Copy, scale=cinvf)
                nc.vector.tensor_add(out=tok[c][:], in0=tok[c][:], in1=t[:])
                nc.vector.tensor_copy(out=tf8[c // 2][:, c % 2, :],
                                      in_=tok[c][:])
            Sf = pgen.tile([1, TOK], f32, tag="g", name="Sf")
            for i in range(4):
                nc.tensor.matmul(Sf[:], ones8p[:], tf8[i // 2][:, i % 2, :],
                                 start=(i == 0), stop=(i == 3))
            Sfb = ac.tile([1, TOK], bf16, tag="rowb", bufs=4, name="Sfb")
            nc.scalar.activation(out=Sfb[:], in_=Sf[:], func=AF.Copy)
            xsqf = []
            for kp in range(2):
                t = ac.tile([128, 2, TOK], bf16, tag="xsq", bufs=4,
                            name=f"xsqf{kp}")
                if kp == 0:
                    nc.vector.tensor_mul(out=t[:], in0=tf8[kp][:],
                                         in1=tf8[kp][:])
                else:
                    nc.scalar.activation(out=t[:], in_=tf8[kp][:],
                                         func=AF.Square)
                xsqf.append(t)
            Qf_ = pgen.tile([1, TOK], f32, tag="g", name="Qf_")
            for i in range(4):
                nc.tensor.matmul(Qf_[:], oi512[:], xsqf[i // 2][:, i % 2, :],
                                 start=(i == 0), stop=(i == 3))
            m2f = ac.tile([1, TOK], f32, tag="rowf", bufs=6, name="m2f")
            nc.scalar.activation(out=m2f[:], in_=Sf[:], func=AF.Square,
                                 scale=1.0 / 64)
            varf = ac.tile([1, TOK], f32, tag="rowf", bufs=6, name="varf")
            nc.vector.tensor_sub(out=varf[:], in0=Qf_[:], in1=m2f[:])
            rstdf = ac.tile([1, TOK], f32, tag="rowf", bufs=6, name="rstdf")
            nc.scalar.activation(out=rstdf[:], in_=varf[:], func=AF.Ln,
                                 bias=epsc[0:1, :])
            nc.scalar.activation(out=rstdf[:], in_=rstdf[:], func=AF.Exp,
                                 scale=-0.5)
            rstdfb = ac.tile([1, TOK], bf16, tag="rowb", bufs=4,
                             name="rstdfb")
            nc.vector.tensor_copy(out=rstdfb[:], in_=rstdf[:])
            bRf_p = pgen.tile([128, TOK], f32, tag="g", name="bRf")
            nc.tensor.matmul(bRf_p[:], ones1[:], rstdfb[:], start=True,
                             stop=True)
            bRf = ac.tile([128, TOK], bf16, tag="bR", bufs=2, name="bRfb")
            nc.vector.tensor_copy(out=bRf[:], in_=bRf_p[:])
            rTf_p = pacc.tile([128, 4], f32, tag="a", name="rTfp")
            for c, (a_, b_) in enumerate(tok_chunks):
                nc.tensor.matmul(rTf_p[0:b_ - a_, c:c + 1],
                                 rstdfb[:, a_:b_],
                                 ones1[0:1, 0:1], start=True, stop=True)
            rTf = ac.tile([128, 4], bf16, tag="rT", bufs=2, name="rTf")
            nc.vector.tensor_copy(out=rTf[:], in_=rTf_p[:])
            rTfs = ac.tile([128, 4], f32, tag="rTs", bufs=2, name="rTfs")
            nc.vector.tensor_copy(out=rTfs[:], in_=rTf_p[:])

            # V_pool + vsums -> pool exchange
            Vp = []
            for i, (a, b) in enumerate(tok_chunks):
                m = b - a
                ps = pacc.tile([128, D], f32, tag="a", name=f"pv{i}")
                for kp in range(2):
                    nc.tensor.matmul(ps[0:m, :], tf8[kp][:, :, a:b],
                                     pwkv8[:, kp, :, D:2 * D],
                                     start=(kp == 0), stop=False,
                                     perf_mode=DR)
                nc.tensor.matmul(ps[0:m, :], Sfb[:, a:b], prow[:, D:2 * D],
                                 start=False, stop=True)
                s = ac.tile([128, D], bf16, tag=f"V{i}", bufs=1,
                            name=f"pVb{i}")
                nc.scalar.activation(out=s[0:m, :], in_=ps[0:m, :],
                                     func=AF.Copy, scale=1.0 / spl)
                Vp.append(s)
            PpA = ac.tile([128, 4], f32, tag="PA", bufs=2, name="PpA")
            for c in range(4):
                ps = pgen.tile([128, 1], f32, tag="g", name=f"pvs{c}")
                for j in range(3):
                    nc.tensor.matmul(ps[:], Vp[j][:, 128 * c:128 * (c + 1)],
                                     rTf[:, j:j + 1], start=(j == 0),
                                     stop=(j == 2))
                nc.vector.tensor_copy(out=PpA[:, c:c + 1], in_=ps[:])
            pvfu = ac.tile([128, 4], f32, tag="vfu", bufs=2, name="pvfu")
            for c in range(4):
                ps = pgen.tile([128, 1], f32, tag="g", name=f"pvf{c}")
                nc.tensor.matmul(ps[:], Vp[3][0:FUS, 128 * c:128 * (c + 1)],
                                 rTf[0:FUS, 3:4], start=True, stop=True)
                nc.vector.tensor_copy(out=pvfu[:, c:c + 1], in_=ps[:])
            pinP = dramp.tile([128, 4], f32, tag="pinA", bufs=2, name="pinP")
            nc.sync.dma_start(out=pinP[:], in_=PpA[:])
            RP = ac.tile([128, 4, 4], f32, tag="RA", bufs=2, name="RP")
            if use_cc:
                poutP = dramp.tile([4 * 128, 4], f32, tag="poutA", bufs=2,
                                   name="poutP")
                nc.gpsimd.collective_compute(
                    "AllGather", OP.bypass, replica_groups=rg,
                    ins=[pinP.opt()], outs=[poutP.opt()])
                nc.sync.dma_start(
                    out=RP[:], in_=poutP.rearrange("(r p) f -> p r f", r=4))
            else:
                nc.sync.dma_start(
                    out=RP[:],
                    in_=pinP.rearrange("(r p) f -> p r f", r=1)
                    .to_broadcast((128, 4, 4)))

            # fusion-key attention for return token 2
            kf = []
            for mc in range(4):
                ps = pgen.tile([128, FUS], f32, tag="g", name=f"pkf{mc}")
                for kp in range(2):
                    nc.tensor.matmul(ps[:],
                                     pwkv8[:, kp, :, 128 * mc:128 * (mc + 1)],
                                     tf8[kp][:, :, OWN:TOK],
                                     start=(kp == 0), stop=False,
                                     perf_mode=DR)
                nc.tensor.matmul(ps[:], prow[:, 128 * mc:128 * (mc + 1)],
                                 Sfb[:, OWN:TOK], start=False, stop=True)
                s = ac.tile([128, FUS], bf16, tag=f"kf{mc}", bufs=1,
                            name=f"kfb{mc}")
                nc.vector.tensor_mul(out=s[:], in0=ps[:],
                                     in1=bRf[:, OWN:TOK])
                kf.append(s)
            q2 = []
            for mc in range(4):
                s = ac.tile([128, 32], bf16, tag=f"qf{mc}", bufs=1,
                            name=f"q2b{mc}")
                nc.vector.memset(s[:, 1:32], 0.0)
                nc.vector.tensor_copy(out=s[:, 0:1], in_=pq2s[mc][:])
                q2.append(s)
            e2, l2 = [], []
            for t in range(2):
                sp = pgen.tile([128, FUS], f32, tag="g", name=f"ps2{t}")
                for i in range(4):
                    h = 4 * t + i
                    ch, base = h // 2, (h % 2) * 64
                    nc.tensor.matmul(sp[32 * i:32 * i + 32, :],
                                     q2[ch][base:base + 64, 0:32],
                                     kf[ch][base:base + 64, :],
                                     start=True, stop=True,
                                     tile_position=(base, 32 * i))
                e = ac.tile([128, FUS], bf16, tag=f"e2{t}", bufs=1,
                            name=f"e2{t}")
                la = ac.tile([128, 1], f32, tag=f"la{t}", bufs=2,
                             name=f"pla{t}")
                nc.scalar.activation(out=e[:], in_=sp[:], func=AF.Exp,
                                     scale=1.0 / spl, accum_out=la[:])
                e2.append(e)
                l2.append(la)
            e2T = []
            for t in range(2):
                pt = pgen.tile([FUS, 128], bf16, tag="g", name=f"pet{t}")
                nc.tensor.transpose(pt[:], e2[t][:], ident[:])
                s = ac.tile([FUS, 128], bf16, tag=f"e2T{t}", bufs=1,
                            name=f"e2Tb{t}")
                nc.vector.tensor_scalar_mul(out=s[:], in0=pt[:],
                                            scalar1=rTfs[0:FUS, 3:4])
                e2T.append(s)
            # uniform pooled vector u
            pvs = ac.tile([128, 4], f32, tag="vsb", bufs=2, name="pvs")
            nc.vector.tensor_reduce(out=pvs[:],
                                    in_=RP[:].rearrange("p r f -> p f r"),
                                    axis=mybir.AxisListType.X,
                                    op=OP.add)
            nc.vector.tensor_add(out=pvs[:], in0=pvs[:], in1=pvfu[:])
            pvsb8 = ac.tile([128, 4], f8, tag="vsb8", bufs=2,
                            name="pvsb8")
            nc.vector.tensor_copy(out=pvsb8[:], in_=pvs[:])
            pup = pgen.tile([128, 4], f32, tag="g", name="pup")
            for c in range(4):
                for kc in range(4):
                    nc.tensor.matmul(pup[:, c:c + 1],
                                     pwo8[:, kc // 2, kc % 2,
                                          128 * c:128 * (c + 1)],
                                     pvsb8[:, kc:kc + 1],
                                     start=(kc == 0), stop=(kc == 3))
            pus = ac.tile([128, 4], f32, tag="dus", bufs=2, name="pus")
            nc.scalar.activation(out=pus[:], in_=pup[:], func=AF.Copy,
                                 scale=1.0 / (spo * NALL))
            nc.sync.dma_start(out=out_u.rearrange("(c p) one -> p (c one)", c=4),
                              in_=pus[:])

            of2 = []
            for t in range(2):
                acc = pacc.tile([128, 64], f32, tag="a", name=f"pacc2{t}")
                for i in range(4):
                    h = 4 * t + i
                    nc.tensor.matmul(acc[32 * i:32 * i + 32, :],
                                     e2T[t][:, 32 * i:32 * i + 32],
                                     Vp[3][0:FUS, DH * h:DH * (h + 1)],
                                     start=True, stop=True,
                                     tile_position=(0, 32 * i))
                li = ac.tile([128, 1], f32, tag="linv", bufs=2,
                             name=f"pli{t}")
                nc.vector.reciprocal(out=li[:], in_=l2[t][:])
                s = ac.tile([128, 64], bf16, tag=f"of{t}", bufs=1,
                            name=f"pof{t}")
                nc.vector.tensor_scalar_mul(out=s[:], in0=acc[:],
                                            scalar1=li[:])
                of2.append(s)
            ofI2 = [ac.tile([128, 1], f8, tag=f"ofI2{kc}", bufs=1,
                            name=f"ofI2{kc}") for kc in range(4)]
            for kc in range(4):
                pt = pacc.tile([128, 32], bf16, tag="a", name=f"ofI2p{kc}")
                for half in range(2):
                    h = 2 * kc + half
                    t, i = h // 4, h % 4
                    nc.tensor.transpose(
                        pt[64 * half:64 * half + 64, :],
                        of2[t][32 * i:32 * i + 32, :],
                        ident[32 * i:32 * i + 32, 32 * i:32 * i + 32],
                        tile_position=(32 * i, 64 * half))
                nc.vector.tensor_copy(out=ofI2[kc][:], in_=pt[:, 0:1])
            P2 = pacc.tile([1, D], f32, tag="a", name="P2")
            for kc in range(4):
                nc.tensor.matmul(P2[:], ofI2[kc][:],
                                 pwo8[:, kc // 2, kc % 2, :],
                                 start=(kc == 0), stop=(kc == 3))
            p2s = ac.tile([1, D], f32, tag="p2s", bufs=1, name="p2s")
            nc.scalar.activation(out=p2s[:], in_=P2[:], func=AF.Copy,
                                 scale=1.0 / spo)
            nc.sync.dma_start(out=out_f, in_=p2s[:])

    nc.compile()
    _built[key] = nc
    return nc


def _prep_inputs(inputs):
    """Host-side prep: fold gains, pick fp8 scales, pack weights per layer."""
    I = {k: np.asarray(v, np.float64) for k, v in inputs.items()}
    f32 = np.float32

    def col(x):
        return np.ascontiguousarray(np.asarray(x, f32).reshape(-1, 1))

    scale_dh = DH ** -0.5
    g = I["layers_attn_g"][:, :, None]
    wqf = I["layers_wq"] * scale_dh * g            # [L, 512, 512]
    wkvf = I["layers_wkv"] * g                     # [L, 512, 1024]
    wof = I["layers_wo"]                           # [L, 512, 512]
    w1f = I["layers_ff_w1"] * I["layers_ff_g"][:, :, None]  # [L, 512, 2730]
    w2f = I["layers_ff_w2"]                        # [L, 1365, 512]

    sq = _pow2_scale(wqf)
    skv = _pow2_scale(wkvf)
    so = _pow2_scale(wof)
    s1x = 16.0
    s1g = _pow2_scale(w1f[:, :, IFF:])
    s2 = _pow2_scale(w2f)
    pkvf = I["pool_wkv"] * I["final_g"][:, None]
    spl = _pow2_scale(pkvf)
    spo = _pow2_scale(I["pool_wo"])
    sef = {}
    for mod in ("rna", "atac"):
        ewf = I[f"{mod}_w"] * I[f"{mod}_ln1_g"][:, None]
        sef[mod] = (ewf, _pow2_scale(ewf))
    se = min(sef["rna"][1], sef["atac"][1])
    scales = (sq, skv, so, s1x, s1g, s2, se, spl, spo)

    # packed per-layer fp8 weights
    wpk = np.zeros((DEPTH, 128, FCOLS), F8)
    rows = np.zeros((DEPTH, 3, D), np.float64)
    for l in range(DEPTH):
        wpk[l, :, SEG_WQ:SEG_WKV] = _pack_pairs(wqf[l], sq).reshape(128, -1)
        wpk[l, :, SEG_WKV:SEG_WO] = _pack_pairs(wkvf[l], skv).reshape(128, -1)
        wpk[l, :, SEG_WO:SEG_W1] = _pack_pairs(wof[l], so).reshape(128, -1)
        w1p = np.zeros((D, 2 * IFFP), np.float64)
        w1p[:, 0:IFF] = w1f[l][:, 0:IFF] * s1x
        w1p[:, IFFP:IFFP + IFF] = w1f[l][:, IFF:] * s1g
        wpk[l, :, SEG_W1:SEG_W2] = _pack_pairs(w1p, 1.0).reshape(128, -1)
        w2p = np.zeros((IFF2, D), np.float64)
        w2p[0:IFF, :] = w2f[l] * s2
        wpk[l, :, SEG_W2:FCOLS] = _pack_pairs(w2p, 1.0).reshape(128, -1)
        # rank-1 rows: -scale*colsum/64
        rows[l, 0] = -skv * wkvf[l][:, 0:D].sum(0) / 64     # wksum
        rows[l, 1] = -skv * wkvf[l][:, D:2 * D].sum(0) / 64  # wvsum
        rows[l, 2] = -sq * wqf[l].sum(0) / 64                # wqsum
    rows_b = np.ascontiguousarray(rows.reshape(1, -1)).astype(BF)

    prow = np.zeros((1, 2 * D), np.float64)
    prow[0, 0:D] = -spl * pkvf[:, 0:D].sum(0) / 64
    prow[0, D:2 * D] = -spl * pkvf[:, D:2 * D].sum(0) / 64

    pwkv8 = _pack_pairs(pkvf, spl)
    pwo8 = _pack_pairs(I["pool_wo"], spo)

    # pool query for return token 2 (host, tiny)
    ret = I["return_tokens"].astype(f32)
    gp = I["pool_g"].astype(f32)
    mu = ret.mean(-1, keepdims=True)
    var = ((ret - mu) ** 2).mean(-1, keepdims=True)
    retn = (ret - mu) / np.sqrt(var + 1e-5) * gp
    q2 = (retn[2] @ I["pool_wq"].astype(f32)) * scale_dh

    fus8 = I["fusion_tokens"].T.reshape(4, 128, FUS).transpose(1, 0, 2)

    shared = {
        "wpk": wpk,
        "rows": rows_b,
        "prow": prow.astype(BF),
        "pwkv8": pwkv8,
        "pwo8": pwo8,
        "pool_q2": col(q2),
        "fus_t": np.ascontiguousarray(fus8).astype(np.float32),
    }

    in_maps = []
    for c in range(N_CORES):
        b, q = c // 4, c % 4
        mod = "rna" if q < 2 else "atac"
        x = I[mod][b, (q % 2) * OWN:(q % 2 + 1) * OWN, :]   # [384, 1024]
        m = dict(shared)
        # x8: [128, 4kp, 2sub, 384]
        xT = np.ascontiguousarray(x.T)                      # [1024, 384]
        x8 = _pack_pairs(xT, 1.0)
        m["x8"] = x8
        ewf = sef[mod][0]
        m["ew8"] = _pack_pairs(ewf, se)
        ecols = np.zeros((128, 4, 3), np.float32)
        ecols[:, :, 0] = (I[f"{mod}_b"]
                          + I[f"{mod}_ln1_b"] @ I[f"{mod}_w"]).reshape(4, 128).T
        ecols[:, :, 1] = I[f"{mod}_ln2_g"].reshape(4, 128).T
        ecols[:, :, 2] = I[f"{mod}_ln2_b"].reshape(4, 128).T
        m["ecols"] = ecols
        # embed input-LN stats from the quantized x8 the device would see
        xq = x8.astype(np.float64).transpose(1, 2, 0, 3).reshape(RIN, OWN)
        Sr = xq.sum(0) / 8.0
        mu = Sr / 128.0
        var = (xq * xq).mean(0) - mu * mu
        rstd = 1.0 / np.sqrt(var + 1e-5)
        erows = np.zeros(D + 2 * OWN, np.float64)
        erows[0:D] = -se * ewf.sum(0) / 128
        erows[D:D + OWN] = Sr
        erows[D + OWN:] = rstd / se
        m["erows"] = erows.reshape(1, -1).astype(BF)
        in_maps.append(m)
    return in_maps, ret.astype(f32), scales


def kernel(**inputs):
    from concourse import bass_utils
    in_maps, ret, scales = _prep_inputs(inputs)
    nc = build(num_devices=N_CORES, use_cc=True, scales=scales)
    res = bass_utils.run_bass_kernel_spmd(nc, in_maps,
                                          core_ids=list(range(N_CORES)))
    out = np.zeros((B, 3, D), np.float32)
    for b in range(2):
        r = res.results[4 * b]
        u = r["out_u"][:, 0]
        f = r["out_f"][0]
        out[b, 0] = u + ret[0]
        out[b, 1] = u + ret[1]
        out[b, 2] = f + ret[2]
    return out


# revision 48
# speedup vs baseline: 1.0064x; 1.0064x over previous
"""BioZorro sparse-attention kernel for 8 Trainium2 NeuronCores.

Sharding: 8 cores = 2 batches x 4 token-quarters (384 own tokens each).
The zorro mask makes all non-fusion query rows fully masked -> uniform
softmax -> their attention output is mean(V); only the 16 fusion tokens
attend (over the 1536 non-fusion keys). Cross-core data per layer is two
small AllGathers: (A) V column sums (2KB) issued early, (B) fusion
flash-softmax partials (66KB), plus one tiny AllGather for pooling.

Compute layout: residual stream feature-major (tok^T [512, 400] f32).
All heavy matmuls run in fp8e4 DoubleRow (contract 256/instr, 2x rate):
activations are cast to paired [128,2,T] tiles; weights are host-packed
into one fp8 buffer per layer (single DMA, double-buffered). LayerNorms
are folded into consumers: raw-cast -> matmul immediately; the -mu
correction enters PSUM as a rank-1 matmul (host-precomputed column sums
x the device S row); rstd is applied at PSUM eviction (column-broadcast
or per-token scalars). Per-tensor power-of-2 fp8 scales are descaled via
free immediate-scale slots (exp/gelu/copy activations).
"""
import sys
sys.path.insert(0, "/opt/trn_rl_repo")
import numpy as np
import ml_dtypes

BF = ml_dtypes.bfloat16
F8 = ml_dtypes.float8_e4m3
OWN, FUS, TOK = 384, 16, 400
D, RIN, H, DH, IFF, DEPTH = 512, 1024, 8, 64, 1365, 4
NALL = 1552
B, NR, NA = 2, 768, 768
N_CORES = 8
IFFP = 1408           # x/gate block padding (11 x 128)
IFF2 = 1536           # FF2 contract padding (6 x 256)

# fp8 packed-weight segment offsets (cols in the per-layer [128, FCOLS])
SEG_WQ = 0            # [2kp][2sub][512]
SEG_WKV = 2048        # [2kp][2sub][1024]
SEG_WO = SEG_WKV + 4096   # [2kp][2sub][512]
SEG_W1 = SEG_WO + 2048    # [2kp][2sub][2*1408]
SEG_W2 = SEG_W1 + 11264   # [6jp][2sub][512]
FCOLS = SEG_W2 + 6144

_built = {}


def _pow2_scale(w, target=120.0):
    m = float(np.abs(w).max())
    if m <= 0:
        return 1.0
    return float(2.0 ** np.floor(np.log2(target / m)))


def _pack_pairs(w, scale):
    """[K, N] f64 -> [128, K//256, 2, N] fp8 DoubleRow lhsT layout."""
    K, N = w.shape
    assert K % 256 == 0
    out = (w * scale).astype(F8).reshape(K // 128, 128, N)
    # chunk k = rows 128k..128k+128; pair kp = (2kp, 2kp+1)
    out = out.transpose(1, 0, 2).reshape(128, K // 256, 2, N)
    return np.ascontiguousarray(out)


def build(num_devices=8, use_cc=True, scales=None):
    key = (num_devices, use_cc, scales)
    if key in _built:
        return _built[key]
    import concourse.tile as tile
    from concourse import bacc, mybir
    from concourse.masks import make_identity

    # Force Exp to resolve to natural_log_exp_and_others so Ln/Exp/Square
    # live in one ACT table set (Gelu still needs its own set; those two
    # swaps per layer are prefetched off the critical path with dummy ops).
    if not getattr(bacc, "_act_tables_patched", False):
        _orig_gat = bacc.get_activation_tables

        def _patched_gat(arch):
            tabs = _orig_gat(arch)
            exp_t = mybir.ActivationFunctionType.Exp
            for nm, fns in tabs.items():
                if nm != "natural_log_exp_and_others":
                    fns.discard(exp_t)
            return tabs

        bacc.get_activation_tables = _patched_gat
        bacc._act_tables_patched = True

    sq, skv, so, s1x, s1g, s2, se, spl, spo = scales
    f32 = mybir.dt.float32
    bf16 = mybir.dt.bfloat16
    f8 = mybir.dt.float8e4
    AF = mybir.ActivationFunctionType
    OP = mybir.AluOpType
    DR = mybir.MatmulPerfMode.DoubleRow

    nc = bacc.Bacc("TRN2", target_bir_lowering=False, debug=False,
                   enable_asserts=True, num_devices=num_devices)

    def din(name, shape, dt=f32):
        return nc.dram_tensor(name, shape, dt, kind="ExternalInput").ap()

    x8_t = din("x8", [128, 4, 2, OWN], f8)
    ew8_t = din("ew8", [128, 4, 2, D], f8)
    ecols_t = din("ecols", [128, 4, 3])
    erows_t = din("erows", [1, D + 2 * OWN], mybir.dt.bfloat16)
    fus_t = din("fus_t", [128, 4, FUS], f32)
    wpk_t = din("wpk", [DEPTH, 128, FCOLS], f8)
    # host rank-1 rows: per layer [wksum, wvsum, wqsum] each [512]
    rows_t = din("rows", [1, DEPTH * 3 * D], bf16)
    prow_t = din("prow", [1, 2 * D], bf16)    # pool [pwksum, pwvsum]
    pwkv8_t = din("pwkv8", [128, 2, 2, 2 * D], f8)
    pwo8_t = din("pwo8", [128, 2, 2, D], f8)
    pq2_t = din("pool_q2", [D, 1])
    out_u = nc.dram_tensor("out_u", [D, 1], f32, kind="ExternalOutput").ap()
    out_f = nc.dram_tensor("out_f", [1, D], f32, kind="ExternalOutput").ap()

    with tile.TileContext(nc) as tc:
        with tc.tile_pool(name="cst", bufs=1) as cst, \
             tc.tile_pool(name="wp", bufs=2) as wp, \
             tc.tile_pool(name="ac", bufs=2) as ac, \
             tc.tile_pool(name="pgen", bufs=4, space="PSUM") as pgen, \
             tc.tile_pool(name="pacc", bufs=4, space="PSUM") as pacc, \
             tc.tile_pool(name="dramp", bufs=2, space="DRAM") as dramp:

            ident = cst.tile([128, 128], bf16, name="ident")
            make_identity(nc, ident[:])
            ones128 = cst.tile([128, 1], bf16, name="ones128")
            nc.vector.memset(ones128[:], 1.0)
            ones1 = cst.tile([1, 128], bf16, name="ones1")
            nc.vector.memset(ones1[:], 1.0)
            epsc = cst.tile([128, 1], f32, name="epsc")
            nc.vector.memset(epsc[:], 1e-5)
            oi512 = cst.tile([128, 1], bf16, name="oi512")
            nc.vector.memset(oi512[:], 1.0 / 512)
            ones8p = cst.tile([128, 1], f8, name="ones8p")
            nc.vector.memset(ones8p[:], 0.125)
            cinvf = 1.0 / (s1x * s2)

            def load_cols(dram_ap, n, tag, rows=128):
                ts = []
                for c in range(n):
                    t = wp.tile([rows, 1], f32, tag=f"{tag}{c}", bufs=1,
                                name=f"{tag}{c}")
                    nc.sync.dma_start(out=t[:],
                                      in_=dram_ap[rows * c:rows * (c + 1), :])
                    ts.append(t)
                return ts

            # ---------- one-time loads (embed inputs first; w0 after) ----------
            x8 = ac.tile([128, 4, 2, OWN], f8, tag="x8", bufs=1, name="x8")
            nc.sync.dma_start(out=x8[:], in_=x8_t)
            ew8 = wp.tile([128, 4, 2, D], f8, tag="ew8", bufs=1, name="ew8")
            nc.sync.dma_start(out=ew8[:], in_=ew8_t)
            erows = wp.tile([1, D + 2 * OWN], bf16, tag="erows", bufs=1,
                            name="erows")
            nc.sync.dma_start(out=erows[:], in_=erows_t)
            ecols = wp.tile([128, 4, 3], f32, tag="ecols", bufs=1,
                            name="ecols")
            nc.sync.dma_start(out=ecols[:], in_=ecols_t)
            rows = wp.tile([1, DEPTH * 3 * D], bf16, tag="rows", bufs=1,
                           name="rows")
            nc.sync.dma_start(out=rows[:], in_=rows_t)
            w0 = wp.tile([128, FCOLS], f8, tag="wpk", bufs=2, name="wpk0")
            nc.sync.dma_start(out=w0[:], in_=wpk_t[0])
            erow = erows[:, 0:D]
            Seb = erows[:, D:D + OWN]
            rstdeb = erows[:, D + OWN:D + 2 * OWN]
            ebs = [ecols[:, c, 0:1] for c in range(4)]
            eg2s = [ecols[:, c, 1:2] for c in range(4)]
            eb2s = [ecols[:, c, 2:3] for c in range(4)]

            # dummy exp to preload the nlexp ACT table during initial DMAs
            dtab = ac.tile([1, 1], f32, tag="dtab", bufs=2, name="dtab")
            nc.scalar.activation(out=dtab[:], in_=epsc[0:1, :], func=AF.Exp)

            # ---------- embed (input-LN stats host-precomputed) ----------
            bRe_p = pgen.tile([128, OWN], f32, tag="g", name="bRe")
            nc.tensor.matmul(bRe_p[:], ones1[:], rstdeb, start=True,
                             stop=True)
            bRe = ac.tile([128, OWN], bf16, tag="bR", bufs=2, name="bReb")
            nc.vector.tensor_copy(out=bRe[:], in_=bRe_p[:])

            hb = []
            for mc in range(4):
                ps = pgen.tile([128, OWN], f32, tag="g", name=f"embp{mc}")
                for kp in range(4):
                    nc.tensor.matmul(ps[:], ew8[:, kp, :, 128 * mc:128 * (mc + 1)],
                                     x8[:, kp, :, :], start=(kp == 0),
                                     stop=False, perf_mode=DR)
                nc.tensor.matmul(ps[:], erow[:, 128 * mc:128 * (mc + 1)],
                                 Seb, start=False, stop=True)
                t1 = ac.tile([128, OWN], bf16, tag="embt", bufs=2,
                             name=f"embt{mc}")
                nc.vector.tensor_mul(out=t1[:], in0=ps[:], in1=bRe[:])
                t2 = ac.tile([128, OWN], bf16, tag=f"hb{mc}", bufs=1,
                             name=f"hb{mc}")
                nc.vector.tensor_scalar_add(out=t2[:], in0=t1[:],
                                            scalar1=ebs[mc])
                hb.append(t2)

            # embed LN2 (explicit normalize into f32 tok)
            S2e = pgen.tile([1, OWN], f32, tag="g", name="S2e")
            for c in range(4):
                nc.tensor.matmul(S2e[:], oi512[:], hb[c][:],
                                 start=(c == 0), stop=(c == 3))
            x2e = []
            for c in range(4):
                t = ac.tile([128, OWN], bf16, tag="xsq", bufs=4,
                            name=f"x2e{c}")
                if c % 2 == 0:
                    nc.vector.tensor_mul(out=t[:], in0=hb[c][:], in1=hb[c][:])
                else:
                    nc.scalar.activation(out=t[:], in_=hb[c][:],
                                         func=AF.Square)
                x2e.append(t)
            Q2e = pgen.tile([1, OWN], f32, tag="g", name="Q2e")
            for c in range(4):
                nc.tensor.matmul(Q2e[:], oi512[:], x2e[c][:],
                                 start=(c == 0), stop=(c == 3))
            m22 = ac.tile([1, OWN], f32, tag="rowf", bufs=6, name="m22")
            nc.scalar.activation(out=m22[:], in_=S2e[:], func=AF.Square)
            var2 = ac.tile([1, OWN], f32, tag="rowf", bufs=6, name="var2e")
            nc.vector.tensor_sub(out=var2[:], in0=Q2e[:], in1=m22[:])
            rstd2e = ac.tile([1, OWN], f32, tag="rowf", bufs=6, name="rstd2e")
            nc.scalar.activation(out=rstd2e[:], in_=var2[:], func=AF.Ln,
                                 bias=epsc[0:1, :])
            nc.scalar.activation(out=rstd2e[:], in_=rstd2e[:], func=AF.Exp,
                                 scale=-0.5)
            pr2 = ac.tile([1, 2 * OWN], bf16, tag="rowb2", bufs=2, name="pr2e")
            nc.vector.tensor_copy(out=pr2[:, 0:OWN], in_=rstd2e[:])
            mre = ac.tile([1, OWN], f32, tag="rowf", bufs=6, name="mre")
            nc.vector.tensor_mul(out=mre[:], in0=S2e[:], in1=rstd2e[:])
            nc.vector.tensor_copy(out=pr2[:, OWN:2 * OWN], in_=mre[:])
            bR2e_p = pgen.tile([128, OWN], f32, tag="g", name="bR2e")
            nc.tensor.matmul(bR2e_p[:], ones1[:], pr2[:, 0:OWN], start=True,
                             stop=True)
            bR2e = ac.tile([128, OWN], bf16, tag="bR", bufs=2, name="bR2eb")
            nc.vector.tensor_copy(out=bR2e[:], in_=bR2e_p[:])
            bM2e_p = pgen.tile([128, OWN], f32, tag="g", name="bM2e")
            nc.tensor.matmul(bM2e_p[:], ones1[:], pr2[:, OWN:2 * OWN],
                             start=True, stop=True)
            bM2e = ac.tile([128, OWN], bf16, tag="bM", bufs=2, name="bM2eb")
            nc.vector.tensor_copy(out=bM2e[:], in_=bM2e_p[:])

            tok = [ac.tile([128, TOK], f32, tag=f"tok{c}", bufs=1,
                           name=f"tok{c}") for c in range(4)]
            for c in range(4):
                t1 = ac.tile([128, OWN], bf16, tag="embt", bufs=2,
                             name=f"eln{c}")
                nc.vector.tensor_mul(out=t1[:], in0=hb[c][:], in1=bR2e[:])
                nc.vector.tensor_sub(out=t1[:], in0=t1[:], in1=bM2e[:])
                nc.vector.tensor_scalar(out=tok[c][:, 0:OWN], in0=t1[:],
                                        scalar1=eg2s[c], scalar2=eb2s[c],
                                        op0=OP.mult, op1=OP.add)
                nc.sync.dma_start(out=tok[c][:, OWN:TOK], in_=fus_t[:, c, :])

            tok_chunks = [(0, 128), (128, 256), (256, 384), (384, 400)]
            rg = [[0, 1, 2, 3], [4, 5, 6, 7]]
            psO_prev = None

            # rank-1 row views per layer
            def lrow(l, which):
                base = (l * 3 + which) * D
                return rows[:, base:base + D]

            # ---------- layers ----------
            for l in range(DEPTH):
                wT = w0 if l == 0 else wp.tile([128, FCOLS], f8, tag="wpk",
                                               bufs=2, name=f"wpk{l}")
                if l > 0:
                    nc.sync.dma_start(out=wT[:], in_=wpk_t[l])
                wq8 = wT[:, SEG_WQ:SEG_WKV].rearrange(
                    "p (kp s o) -> p kp s o", kp=2, s=2)
                wkv8 = wT[:, SEG_WKV:SEG_WO].rearrange(
                    "p (kp s o) -> p kp s o", kp=2, s=2)
                wo8 = wT[:, SEG_WO:SEG_W1].rearrange(
                    "p (kp s o) -> p kp s o", kp=2, s=2)
                w18 = wT[:, SEG_W1:SEG_W2].rearrange(
                    "p (kp s o) -> p kp s o", kp=2, s=2)
                w28 = wT[:, SEG_W2:FCOLS].rearrange(
                    "p (jp s o) -> p jp s o", jp=6, s=2)

                # --- LN1 prep: residual + fp8 raw cast ---
                tr8 = [ac.tile([128, 2, TOK], f8, tag=f"tr8{kp}", bufs=1,
                               name=f"tr8_{kp}") for kp in range(2)]
                for c in range(4):
                    if psO_prev is not None:
                        t = ac.tile([128, TOK], bf16, tag="psot", bufs=2,
                                    name=f"psot{c}")
                        nc.scalar.activation(out=t[:], in_=psO_prev[c][:],
                                             func=AF.Copy, scale=cinvf)
                        nc.vector.tensor_add(out=tok[c][:], in0=tok[c][:],
                                             in1=t[:])
                    nc.vector.tensor_copy(out=tr8[c // 2][:, c % 2, :],
                                          in_=tok[c][:])
                psO_prev = None

                # --- stats1 (rank-1 style folded LN) ---
                S1 = pgen.tile([1, TOK], f32, tag="g", name="S1")
                for i in range(4):
                    nc.tensor.matmul(S1[:], ones8p[:], tr8[i // 2][:, i % 2, :],
                                     start=(i == 0), stop=(i == 3))
                S1b = ac.tile([1, TOK], bf16, tag="rowb", bufs=4, name="S1b")
                nc.scalar.activation(out=S1b[:], in_=S1[:], func=AF.Copy)
                m2 = ac.tile([1, TOK], f32, tag="rowf", bufs=6, name="m2")
                nc.scalar.activation(out=m2[:], in_=S1[:], func=AF.Square,
                                     scale=1.0 / 64)
                xsq = []
                for kp in range(2):
                    t = ac.tile([128, 2, TOK], bf16, tag="xsq", bufs=4,
                                name=f"xsq{kp}")
                    for sub in range(2):
                        if (2 * kp + sub) % 2 == 0:
                            nc.vector.tensor_mul(out=t[:, sub, :],
                                                 in0=tr8[kp][:, sub, :],
                                                 in1=tr8[kp][:, sub, :])
                        else:
                            nc.scalar.activation(out=t[:, sub, :],
                                                 in_=tr8[kp][:, sub, :],
                                                 func=AF.Square)
                    xsq.append(t)

                # --- V raw (+ early evict, rstd folded into consumers) ---
                V = []
                for i, (a, b) in enumerate(tok_chunks):
                    m = b - a
                    ps = pacc.tile([128, D], f32, tag="a", name=f"v{i}")
                    for kp in range(2):
                        nc.tensor.matmul(ps[0:m, :], tr8[kp][:, :, a:b],
                                         wkv8[:, kp, :, D:2 * D],
                                         start=(kp == 0), stop=False,
                                         perf_mode=DR)
                    nc.tensor.matmul(ps[0:m, :], S1b[:, a:b],
                                     lrow(l, 1), start=False, stop=True)
                    sV = ac.tile([128, D], bf16, tag=f"V{i}", bufs=1,
                                 name=f"Vb{i}")
                    nc.scalar.activation(out=sV[0:m, :], in_=ps[0:m, :],
                                         func=AF.Copy, scale=1.0 / skv)
                    V.append(sV)

                # --- K^T, Qf raw matmuls ---
                ktp, qfp = [], []
                for mc in range(4):
                    ps = pacc.tile([128, OWN], f32, tag="a", name=f"kt{mc}")
                    for kp in range(2):
                        nc.tensor.matmul(ps[:],
                                         wkv8[:, kp, :, 128 * mc:128 * (mc + 1)],
                                         tr8[kp][:, :, 0:OWN],
                                         start=(kp == 0), stop=False,
                                         perf_mode=DR)
                    nc.tensor.matmul(ps[:], lrow(l, 0)[:, 128 * mc:128 * (mc + 1)],
                                     S1b[:, 0:OWN], start=False, stop=True)
                    ktp.append(ps)
                # --- stats1 tail ---
                Q1 = pgen.tile([1, TOK], f32, tag="g", name="Q1")
                for i in range(4):
                    nc.tensor.matmul(Q1[:], oi512[:], xsq[i // 2][:, i % 2, :],
                                     start=(i == 0), stop=(i == 3))
                var = ac.tile([1, TOK], f32, tag="rowf", bufs=6, name="var")
                nc.vector.tensor_sub(out=var[:], in0=Q1[:], in1=m2[:])
                rstd = ac.tile([1, TOK], f32, tag="rowf", bufs=6, name="rstd")
                nc.scalar.activation(out=rstd[:], in_=var[:], func=AF.Ln,
                                     bias=epsc[0:1, :])
                nc.scalar.activation(out=rstd[:], in_=rstd[:], func=AF.Exp,
                                     scale=-0.5)
                rstdb = ac.tile([1, TOK], bf16, tag="rowb", bufs=4,
                                name="rstdb")
                nc.vector.tensor_copy(out=rstdb[:], in_=rstd[:])
                bR1_p = pgen.tile([128, TOK], f32, tag="g", name="bR1")
                nc.tensor.matmul(bR1_p[:], ones1[:], rstdb[:], start=True,
                                 stop=True)
                bR1 = ac.tile([128, TOK], bf16, tag="bR", bufs=2, name="bR1b")
                nc.vector.tensor_copy(out=bR1[:], in_=bR1_p[:])
                rT_p = pgen.tile([128, 4], f32, tag="g", name="rTp")
                for c, (a_, b_) in enumerate(tok_chunks):
                    nc.tensor.matmul(rT_p[0:b_ - a_, c:c + 1],
                                     rstdb[:, a_:b_],
                                     ones1[0:1, 0:1], start=True, stop=True)
                rT = ac.tile([128, 4], bf16, tag="rT", bufs=2, name="rT")
                nc.vector.tensor_copy(out=rT[:], in_=rT_p[:])
                rTs = ac.tile([128, 4], f32, tag="rTs", bufs=2, name="rTs")
                nc.vector.tensor_copy(out=rTs[:], in_=rT_p[:])

                # --- Qf raw (after stats-tail matmuls; "a" slots free
                # once kt evictions land) ---
                for mc in range(4):
                    ps = pacc.tile([128, FUS], f32, tag="a", name=f"qf{mc}")
                    for kp in range(2):
                        nc.tensor.matmul(ps[:],
                                         wq8[:, kp, :, 128 * mc:128 * (mc + 1)],
                                         tr8[kp][:, :, OWN:TOK],
                                         start=(kp == 0), stop=False,
                                         perf_mode=DR)
                    nc.tensor.matmul(ps[:], lrow(l, 2)[:, 128 * mc:128 * (mc + 1)],
                                     S1b[:, OWN:TOK], start=False, stop=True)
                    qfp.append(ps)

                # --- vsums (rstd via rhs) -> exchange A ---
                PA = ac.tile([128, 4], f32, tag="PA", bufs=2, name="PA")
                for c in range(4):
                    ps = pgen.tile([128, 1], f32, tag="g", name=f"vs{c}")
                    for j in range(3):
                        nc.tensor.matmul(ps[:], V[j][:, 128 * c:128 * (c + 1)],
                                         rT[:, j:j + 1], start=(j == 0),
                                         stop=(j == 2))
                    nc.vector.tensor_copy(out=PA[:, c:c + 1], in_=ps[:])
                vfu = ac.tile([128, 4], f32, tag="vfu", bufs=2, name="vfu")
                for c in range(4):
                    ps = pgen.tile([128, 1], f32, tag="g", name=f"vf{c}")
                    nc.tensor.matmul(ps[:], V[3][0:FUS, 128 * c:128 * (c + 1)],
                                     rT[0:FUS, 3:4], start=True, stop=True)
                    nc.vector.tensor_copy(out=vfu[:, c:c + 1], in_=ps[:])
                pinA = dramp.tile([128, 4], f32, tag="pinA", bufs=2,
                                  name="pinA")
                nc.sync.dma_start(out=pinA[:], in_=PA[:])
                RA = ac.tile([128, 4, 4], f32, tag="RA", bufs=2, name="RA")
                if use_cc:
                    poutA = dramp.tile([4 * 128, 4], f32, tag="poutA", bufs=2,
                                       name="poutA")
                    nc.gpsimd.collective_compute(
                        "AllGather", OP.bypass, replica_groups=rg,
                        ins=[pinA.opt()], outs=[poutA.opt()])
                    nc.sync.dma_start(
                        out=RA[:], in_=poutA.rearrange("(r p) f -> p r f", r=4))
                else:
                    nc.sync.dma_start(
                        out=RA[:],
                        in_=pinA.rearrange("(r p) f -> p r f", r=1)
                        .to_broadcast((128, 4, 4)))

                # --- kt/qf evictions (x rstd columns) ---
                kt = []
                for mc in range(4):
                    sK = ac.tile([128, OWN], bf16, tag=f"kt{mc}", bufs=1,
                                 name=f"ktb{mc}")
                    nc.vector.tensor_mul(out=sK[:], in0=ktp[mc][:],
                                         in1=bR1[:, 0:OWN])
                    kt.append(sK)
                qf = []
                for mc in range(4):
                    sQ = ac.tile([128, 32], bf16, tag=f"qf{mc}", bufs=1,
                                 name=f"qfb{mc}")
                    nc.vector.memset(sQ[:, FUS:32], 0.0)
                    nc.vector.tensor_mul(out=sQ[:, 0:FUS], in0=qfp[mc][:],
                                         in1=bR1[:, OWN:TOK])
                    qf.append(sQ)

                # --- scores + exp ---
                E, lacc = [], []
                for t in range(2):
                    sp = pgen.tile([128, OWN], f32, tag="g", name=f"sp{t}")
                    for i in range(4):
                        h = 4 * t + i
                        ch, base = h // 2, (h % 2) * 64
                        nc.tensor.matmul(sp[32 * i:32 * i + 32, :],
                                         qf[ch][base:base + 64, 0:32],
                                         kt[ch][base:base + 64, :],
                                         start=True, stop=True,
                                         tile_position=(base, 32 * i))
                    e = ac.tile([128, OWN], bf16, tag=f"e{t}", bufs=1,
                                name=f"e{t}")
                    la = ac.tile([128, 1], f32, tag=f"la{t}", bufs=2,
                                 name=f"la{t}")
                    nc.scalar.activation(out=e[:], in_=sp[:], func=AF.Exp,
                                         scale=1.0 / (sq * skv),
                                         accum_out=la[:])
                    E.append(e)
                    lacc.append(la)

                # --- E^T + ACC -> exchange B ---
                ET = [[None] * 3 for _ in range(2)]
                for t in range(2):
                    for j in range(3):
                        pt = pgen.tile([128, 128], bf16, tag="g",
                                       name=f"et{t}{j}")
                        nc.tensor.transpose(pt[:], E[t][:, 128 * j:128 * (j + 1)],
                                            ident[:])
                        s = ac.tile([128, 128], bf16, tag=f"ET{t}{j}", bufs=1,
                                    name=f"ETb{t}{j}")
                        nc.vector.tensor_scalar_mul(out=s[:], in0=pt[:],
                                                    scalar1=rTs[:, j:j + 1])
                        ET[t][j] = s
                PB = ac.tile([128, 130], f32, tag="PB", bufs=2, name="PB")
                nc.vector.tensor_copy(out=PB[:, 0:1], in_=lacc[0][:])
                nc.vector.tensor_copy(out=PB[:, 1:2], in_=lacc[1][:])
                for t in range(2):
                    acc = pacc.tile([128, 64], f32, tag="a", name=f"acc{t}")
                    for i in range(4):
                        h = 4 * t + i
                        for j in range(3):
                            nc.tensor.matmul(acc[32 * i:32 * i + 32, :],
                                             ET[t][j][:, 32 * i:32 * i + 32],
                                             V[j][:, DH * h:DH * (h + 1)],
                                             start=(j == 0), stop=(j == 2),
                                             tile_position=(0, 32 * i))
                    nc.vector.tensor_copy(out=PB[:, 2 + 64 * t:66 + 64 * t],
                                          in_=acc[:])
                pinB = dramp.tile([128, 130], f32, tag="pinB", bufs=2,
                                  name="pinB")
                nc.sync.dma_start(out=pinB[:], in_=PB[:])
                RB = ac.tile([128, 4, 130], f32, tag="RB", bufs=2, name="RB")
                if use_cc:
                    poutB = dramp.tile([4 * 128, 130], f32, tag="poutB",
                                       bufs=2, name="poutB")
                    nc.gpsimd.collective_compute(
                        "AllGather", OP.bypass, replica_groups=rg,
                        ins=[pinB.opt()], outs=[poutB.opt()])
                    nc.sync.dma_start(
                        out=RB[:], in_=poutB.rearrange("(r p) f -> p r f", r=4))
                else:
                    nc.sync.dma_start(
                        out=RB[:],
                        in_=pinB.rearrange("(r p) f -> p r f", r=1)
                        .to_broadcast((128, 4, 130)))

                # --- A-combine -> uniform delta -> own-col LN2 prep ---
                vsb = ac.tile([128, 4], f32, tag="vsb", bufs=2, name="vsb")
                nc.vector.tensor_reduce(out=vsb[:],
                                        in_=RA[:].rearrange("p r f -> p f r"),
                                        axis=mybir.AxisListType.X,
                                        op=OP.add)
                nc.vector.tensor_add(out=vsb[:], in0=vsb[:], in1=vfu[:])
                vsb8 = ac.tile([128, 2, 2], f8, tag="vsb8", bufs=2,
                               name="vsb8")
                nc.vector.tensor_copy(
                    out=vsb8[:],
                    in_=vsb[:].rearrange("p (kp s) -> p s kp", kp=2))
                dup = pgen.tile([128, 4], f32, tag="g", name="dup")
                for c in range(4):
                    for kp in range(2):
                        nc.tensor.matmul(dup[:, c:c + 1],
                                         wo8[:, kp, :, 128 * c:128 * (c + 1)],
                                         vsb8[:, :, kp:kp + 1],
                                         start=(kp == 0), stop=(kp == 1),
                                         perf_mode=DR)
                dus = ac.tile([128, 4], f32, tag="dus", bufs=2, name="dus")
                nc.scalar.activation(out=dus[:], in_=dup[:], func=AF.Copy,
                                     scale=1.0 / (so * NALL))
                t28 = [ac.tile([128, 2, TOK], f8, tag=f"t28_{kp}", bufs=1,
                               name=f"t28_{kp}") for kp in range(2)]
                for c in range(4):
                    nc.vector.tensor_scalar_add(out=tok[c][:, 0:OWN],
                                                in0=tok[c][:, 0:OWN],
                                                scalar1=dus[:, c:c + 1])
                    nc.vector.tensor_copy(out=t28[c // 2][:, c % 2, 0:OWN],
                                          in_=tok[c][:, 0:OWN])
                S2 = pgen.tile([1, OWN], f32, tag="g", name="S2")
                for i in range(4):
                    nc.tensor.matmul(S2[:], ones8p[:],
                                     t28[i // 2][:, i % 2, 0:OWN],
                                     start=(i == 0), stop=(i == 3))
                mur2b = ac.tile([1, TOK], bf16, tag="rowb", bufs=4,
                                name="mur2b")
                nc.scalar.activation(out=mur2b[:, 0:OWN], in_=S2[:],
                                     func=AF.Copy, scale=1.0 / 64)
                m2_2 = ac.tile([1, TOK], f32, tag="rowf", bufs=6, name="m2_2")
                nc.scalar.activation(out=m2_2[:, 0:OWN], in_=S2[:],
                                     func=AF.Square, scale=1.0 / 64)
                bMu_p = pgen.tile([128, OWN], f32, tag="g", name="bMu")
                nc.tensor.matmul(bMu_p[:], ones1[:], mur2b[:, 0:OWN],
                                 start=True, stop=True)
                bMu = ac.tile([128, TOK], bf16, tag="bM", bufs=2, name="bMub")
                nc.vector.tensor_copy(out=bMu[:, 0:OWN], in_=bMu_p[:])
                for c in range(4):
                    nc.vector.tensor_sub(out=tok[c][:, 0:OWN],
                                         in0=tok[c][:, 0:OWN],
                                         in1=bMu[:, 0:OWN])
                xsq2 = []
                for kp in range(2):
                    t = ac.tile([128, 2, TOK], bf16, tag="xsq", bufs=4,
                                name=f"xsq2{kp}")
                    for sub in range(2):
                        if (2 * kp + sub) % 2 == 0:
                            nc.vector.tensor_mul(out=t[:, sub, 0:OWN],
                                                 in0=t28[kp][:, sub, 0:OWN],
                                                 in1=t28[kp][:, sub, 0:OWN])
                        else:
                            nc.scalar.activation(out=t[:, sub, 0:OWN],
                                                 in_=t28[kp][:, sub, 0:OWN],
                                                 func=AF.Square)
                    xsq2.append(t)
                Q2 = pgen.tile([1, OWN], f32, tag="g", name="Q2")
                for i in range(4):
                    nc.tensor.matmul(Q2[:], oi512[:],
                                     xsq2[i // 2][:, i % 2, 0:OWN],
                                     start=(i == 0), stop=(i == 3))

                # --- own-col stats2 tail (overlaps exchange B) ---
                var2 = ac.tile([1, TOK], f32, tag="rowf", bufs=6, name="var2")
                nc.vector.tensor_sub(out=var2[:, 0:OWN], in0=Q2[:],
                                     in1=m2_2[:, 0:OWN])
                rstd2 = ac.tile([1, TOK], f32, tag="rowf", bufs=6,
                                name="rstd2")
                nc.scalar.activation(out=rstd2[:, 0:OWN], in_=var2[:, 0:OWN],
                                     func=AF.Ln, bias=epsc[0:1, :])
                nc.scalar.activation(out=rstd2[:, 0:OWN], in_=rstd2[:, 0:OWN],
                                     func=AF.Exp, scale=-0.5)
                rstd2b = ac.tile([1, TOK], bf16, tag="rowb", bufs=4,
                                 name="rstd2b")
                nc.vector.tensor_copy(out=rstd2b[:, 0:OWN],
                                      in_=rstd2[:, 0:OWN])
                bR2_p = pgen.tile([128, OWN], f32, tag="g", name="bR2")
                nc.tensor.matmul(bR2_p[:], ones1[:], rstd2b[:, 0:OWN],
                                 start=True, stop=True)
                bR2 = ac.tile([128, TOK], bf16, tag="bR", bufs=2, name="bR2b")
                nc.vector.tensor_copy(out=bR2[:, 0:OWN], in_=bR2_p[:])
                xc8 = [ac.tile([128, 2, TOK], f8, tag=f"xc8_{kp}", bufs=1,
                               name=f"xc8_{kp}") for kp in range(2)]
                for c in range(4):
                    nc.vector.tensor_mul(out=xc8[c // 2][:, c % 2, 0:OWN],
                                         in0=tok[c][:, 0:OWN],
                                         in1=bR2[:, 0:OWN])

                # --- B-combine -> fusion delta -> fusion-col LN2 prep ---
                PT = ac.tile([128, 130], f32, tag="cmbB", bufs=3, name="PT")
                nc.vector.tensor_reduce(out=PT[:],
                                        in_=RB[:].rearrange("p r f -> p f r"),
                                        axis=mybir.AxisListType.X,
                                        op=OP.add)
                linv = ac.tile([128, 2], f32, tag="linv", bufs=2, name="linv")
                nc.vector.reciprocal(out=linv[:], in_=PT[:, 0:2])
                of = []
                for t in range(2):
                    s = ac.tile([128, 64], bf16, tag=f"of{t}", bufs=1,
                                name=f"of{t}")
                    nc.vector.tensor_scalar_mul(out=s[:],
                                                in0=PT[:, 2 + 64 * t:66 + 64 * t],
                                                scalar1=linv[:, t:t + 1])
                    of.append(s)
                ofI8 = [ac.tile([128, 2, FUS], f8, tag=f"ofI{kp}", bufs=1,
                                name=f"ofI{kp}") for kp in range(2)]
                for kc in range(4):
                    pt = pacc.tile([128, 32], bf16, tag="a", name=f"ofIp{kc}")
                    for half in range(2):
                        h = 2 * kc + half
                        t, i = h // 4, h % 4
                        nc.tensor.transpose(
                            pt[64 * half:64 * half + 64, :],
                            of[t][32 * i:32 * i + 32, :],
                            ident[32 * i:32 * i + 32, 32 * i:32 * i + 32],
                            tile_position=(32 * i, 64 * half))
                    nc.vector.tensor_copy(out=ofI8[kc // 2][:, kc % 2, :],
                                          in_=pt[:, 0:FUS])
                for c in range(4):
                    dfp = pacc.tile([128, FUS], f32, tag="a", name=f"dfp{c}")
                    for kp in range(2):
                        nc.tensor.matmul(dfp[:],
                                         wo8[:, kp, :, 128 * c:128 * (c + 1)],
                                         ofI8[kp][:],
                                         start=(kp == 0), stop=(kp == 1),
                                         perf_mode=DR)
                    dft = ac.tile([128, FUS], bf16, tag="dft", bufs=2,
                                  name=f"dftb{c}")
                    nc.scalar.activation(out=dft[:], in_=dfp[:],
                                         func=AF.Copy, scale=1.0 / so)
                    nc.vector.tensor_add(out=tok[c][:, OWN:TOK],
                                         in0=tok[c][:, OWN:TOK], in1=dft[:])
                    nc.vector.tensor_copy(out=t28[c // 2][:, c % 2, OWN:TOK],
                                          in_=tok[c][:, OWN:TOK])
                S2f = pgen.tile([1, FUS], f32, tag="g", name="S2f")
                for i in range(4):
                    nc.tensor.matmul(S2f[:], ones8p[:],
                                     t28[i // 2][:, i % 2, OWN:TOK],
                                     start=(i == 0), stop=(i == 3))
                for kp in range(2):
                    nc.scalar.activation(out=xsq2[kp][:, :, OWN:TOK],
                                         in_=t28[kp][:, :, OWN:TOK],
                                         func=AF.Square)
                Q2f = pgen.tile([1, FUS], f32, tag="g", name="Q2f")
                for i in range(4):
                    nc.tensor.matmul(Q2f[:], oi512[:],
                                     xsq2[i // 2][:, i % 2, OWN:TOK],
                                     start=(i == 0), stop=(i == 3))

                # --- fusion-col centering + tail (post exchange B) ---
                nc.scalar.activation(out=mur2b[:, OWN:TOK], in_=S2f[:],
                                     func=AF.Copy, scale=1.0 / 64)
                bMuf_p = pgen.tile([128, FUS], f32, tag="g", name="bMuf")
                nc.tensor.matmul(bMuf_p[:], ones1[:],
                                 mur2b[:, OWN:TOK], start=True, stop=True)
                nc.vector.tensor_copy(out=bMu[:, OWN:TOK],
                                      in_=bMuf_p[:])
                for c in range(4):
                    nc.vector.tensor_sub(out=tok[c][:, OWN:TOK],
                                         in0=tok[c][:, OWN:TOK],
                                         in1=bMu[:, OWN:TOK])
                nc.scalar.activation(out=m2_2[:, OWN:TOK], in_=S2f[:],
                                     func=AF.Square, scale=1.0 / 64)
                nc.vector.tensor_sub(out=var2[:, OWN:TOK], in0=Q2f[:],
                                     in1=m2_2[:, OWN:TOK])
                nc.scalar.activation(out=rstd2[:, OWN:TOK],
                                     in_=var2[:, OWN:TOK],
                                     func=AF.Ln, bias=epsc[0:1, :])
                nc.scalar.activation(out=rstd2[:, OWN:TOK],
                                     in_=rstd2[:, OWN:TOK],
                                     func=AF.Exp, scale=-0.5)
                nc.vector.tensor_copy(out=rstd2b[:, OWN:TOK],
                                      in_=rstd2[:, OWN:TOK])
                bR2f_p = pgen.tile([128, FUS], f32, tag="g", name="bR2f")
                nc.tensor.matmul(bR2f_p[:], ones1[:],
                                 rstd2b[:, OWN:TOK], start=True, stop=True)
                nc.vector.tensor_copy(out=bR2[:, OWN:TOK],
                                      in_=bR2f_p[:])
                # dummy gelu: preload the gelu table while FF1 matmuls run
                dt2 = ac.tile([1, 1], f32, tag="dtab", bufs=2, name="dt2")
                nc.scalar.activation(out=dt2[:], in_=rstd2b[0:1, 0:1],
                                     func=AF.Gelu)
                for c in range(4):
                    nc.vector.tensor_mul(out=xc8[c // 2][:, c % 2, OWN:TOK],
                                         in0=tok[c][:, OWN:TOK],
                                         in1=bR2[:, OWN:TOK])

                # --- FF1 / GEGLU / FF2 ---
                gt8 = [ac.tile([128, 2, TOK], f8, tag=f"gt{jp}", bufs=1,
                               name=f"gt{jp}") for jp in range(6)]
                nc.vector.memset(gt8[5][:, 1, :], 0.0)
                for j in range(11):
                    px = pgen.tile([128, TOK], f32, tag="g", name=f"fx{j}")
                    pg = pacc.tile([128, TOK], f32, tag="a", name=f"fg{j}")
                    for kp in range(2):
                        nc.tensor.matmul(px[:], w18[:, kp, :, 128 * j:128 * (j + 1)],
                                         xc8[kp][:], start=(kp == 0),
                                         stop=(kp == 1), perf_mode=DR)
                    for kp in range(2):
                        nc.tensor.matmul(
                            pg[:],
                            w18[:, kp, :, IFFP + 128 * j:IFFP + 128 * (j + 1)],
                            xc8[kp][:], start=(kp == 0),
                            stop=(kp == 1), perf_mode=DR)
                    gg = ac.tile([128, TOK], bf16, tag="gg", bufs=3,
                                 name=f"gg{j}")
                    nc.scalar.activation(out=gg[:], in_=pg[:], func=AF.Gelu,
                                         scale=1.0 / s1g)
                    nc.vector.tensor_mul(out=gt8[j // 2][:, j % 2, :],
                                         in0=gg[:], in1=px[:])
                # dummy exp: preload nlexp for the next layer during FF2
                dt3 = ac.tile([1, 1], f32, tag="dtab", bufs=2, name="dt3")
                nc.scalar.activation(out=dt3[:], in_=gt8[5][0:1, 0, 0:1],
                                     func=AF.Exp)
                psO_prev = []
                for c in range(4):
                    psO = pacc.tile([128, TOK], f32, tag="a", name=f"fo{c}")
                    for jp in range(6):
                        nc.tensor.matmul(psO[:], w28[:, jp, :, 128 * c:128 * (c + 1)],
                                         gt8[jp][:], start=(jp == 0),
                                         stop=(jp == 5), perf_mode=DR)
                    psO_prev.append(psO)

            # ---------- pool ----------
            pwkv8 = wp.tile([128, 2, 2, 2 * D], f8, tag="pwkv8", bufs=1,
                            name="pwkv8")
            nc.sync.dma_start(out=pwkv8[:], in_=pwkv8_t)
            pwo8 = wp.tile([128, 2, 2, D], f8, tag="pwo8", bufs=1,
                           name="pwo8")
            nc.sync.dma_start(out=pwo8[:], in_=pwo8_t)
            prow = wp.tile([1, 2 * D], bf16, tag="prow", bufs=1, name="prow")
            nc.sync.dma_start(out=prow[:], in_=prow_t)
            pq2s = load_cols(pq2_t, 4, "pq2")

            # final LN (rank-1 folded) over tok + last FF residual
            tf8 = [ac.tile([128, 2, TOK], f8, tag=f"tr8{kp}", bufs=1,
                           name=f"tf8_{kp}") for kp in range(2)]
            for c in range(4):
                t = ac.tile([128, TOK], bf16, tag="psot", bufs=2,
                            name=f"fpsot{c}")
                nc.scalar.activation(out=t[:], in_=psO_prev[c][:],
                                     func=AF.Copy, scale=cinvf)
                nc.vector.tensor_add(out=tok[c][:], in0=tok[c][:], in1=t[:])
                nc.vector.tensor_copy(out=tf8[c // 2][:, c % 2, :],
                                      in_=tok[c][:])
            Sf = pgen.tile([1, TOK], f32, tag="g", name="Sf")
            for i in range(4):
                nc.tensor.matmul(Sf[:], ones8p[:], tf8[i // 2][:, i % 2, :],
                                 start=(i == 0), stop=(i == 3))
            Sfb = ac.tile([1, TOK], bf16, tag="rowb", bufs=4, name="Sfb")
            nc.scalar.activation(out=Sfb[:], in_=Sf[:], func=AF.Copy)
            xsqf = []
            for kp in range(2):
                t = ac.tile([128, 2, TOK], bf16, tag="xsq", bufs=4,
                            name=f"xsqf{kp}")
                for sub in range(2):
                    if (2 * kp + sub) % 2 == 0:
                        nc.vector.tensor_mul(out=t[:, sub, :],
                                             in0=tf8[kp][:, sub, :],
                                             in1=tf8[kp][:, sub, :])
                    else:
                        nc.scalar.activation(out=t[:, sub, :],
                                             in_=tf8[kp][:, sub, :],
                                             func=AF.Square)
                xsqf.append(t)
            Qf_ = pgen.tile([1, TOK], f32, tag="g", name="Qf_")
            for i in range(4):
                nc.tensor.matmul(Qf_[:], oi512[:], xsqf[i // 2][:, i % 2, :],
                                 start=(i == 0), stop=(i == 3))
            m2f = ac.tile([1, TOK], f32, tag="rowf", bufs=6, name="m2f")
            nc.scalar.activation(out=m2f[:], in_=Sf[:], func=AF.Square,
                                 scale=1.0 / 64)
            varf = ac.tile([1, TOK], f32, tag="rowf", bufs=6, name="varf")
            nc.vector.tensor_sub(out=varf[:], in0=Qf_[:], in1=m2f[:])
            rstdf = ac.tile([1, TOK], f32, tag="rowf", bufs=6, name="rstdf")
            nc.scalar.activation(out=rstdf[:], in_=varf[:], func=AF.Ln,
                                 bias=epsc[0:1, :])
            nc.scalar.activation(out=rstdf[:], in_=rstdf[:], func=AF.Exp,
                                 scale=-0.5)
            rstdfb = ac.tile([1, TOK], bf16, tag="rowb", bufs=4,
                             name="rstdfb")
            nc.vector.tensor_copy(out=rstdfb[:], in_=rstdf[:])
            bRf_p = pgen.tile([128, TOK], f32, tag="g", name="bRf")
            nc.tensor.matmul(bRf_p[:], ones1[:], rstdfb[:], start=True,
                             stop=True)
            bRf = ac.tile([128, TOK], bf16, tag="bR", bufs=2, name="bRfb")
            nc.vector.tensor_copy(out=bRf[:], in_=bRf_p[:])
            rTf_p = pacc.tile([128, 4], f32, tag="a", name="rTfp")
            for c, (a_, b_) in enumerate(tok_chunks):
                nc.tensor.matmul(rTf_p[0:b_ - a_, c:c + 1],
                                 rstdfb[:, a_:b_],
                                 ones1[0:1, 0:1], start=True, stop=True)
            rTf = ac.tile([128, 4], bf16, tag="rT", bufs=2, name="rTf")
            nc.vector.tensor_copy(out=rTf[:], in_=rTf_p[:])
            rTfs = ac.tile([128, 4], f32, tag="rTs", bufs=2, name="rTfs")
            nc.vector.tensor_copy(out=rTfs[:], in_=rTf_p[:])

            # V_pool + vsums -> pool exchange
            Vp = []
            for i, (a, b) in enumerate(tok_chunks):
                m = b - a
                ps = pacc.tile([128, D], f32, tag="a", name=f"pv{i}")
                for kp in range(2):
                    nc.tensor.matmul(ps[0:m, :], tf8[kp][:, :, a:b],
                                     pwkv8[:, kp, :, D:2 * D],
                                     start=(kp == 0), stop=False,
                                     perf_mode=DR)
                nc.tensor.matmul(ps[0:m, :], Sfb[:, a:b], prow[:, D:2 * D],
                                 start=False, stop=True)
                s = ac.tile([128, D], bf16, tag=f"V{i}", bufs=1,
                            name=f"pVb{i}")
                nc.scalar.activation(out=s[0:m, :], in_=ps[0:m, :],
                                     func=AF.Copy, scale=1.0 / spl)
                Vp.append(s)
            PpA = ac.tile([128, 4], f32, tag="PA", bufs=2, name="PpA")
            for c in range(4):
                ps = pgen.tile([128, 1], f32, tag="g", name=f"pvs{c}")
                for j in range(3):
                    nc.tensor.matmul(ps[:], Vp[j][:, 128 * c:128 * (c + 1)],
                                     rTf[:, j:j + 1], start=(j == 0),
                                     stop=(j == 2))
                nc.vector.tensor_copy(out=PpA[:, c:c + 1], in_=ps[:])
            pvfu = ac.tile([128, 4], f32, tag="vfu", bufs=2, name="pvfu")
            for c in range(4):
                ps = pgen.tile([128, 1], f32, tag="g", name=f"pvf{c}")
                nc.tensor.matmul(ps[:], Vp[3][0:FUS, 128 * c:128 * (c + 1)],
                                 rTf[0:FUS, 3:4], start=True, stop=True)
                nc.vector.tensor_copy(out=pvfu[:, c:c + 1], in_=ps[:])
            pinP = dramp.tile([128, 4], f32, tag="pinA", bufs=2, name="pinP")
            nc.sync.dma_start(out=pinP[:], in_=PpA[:])
            RP = ac.tile([128, 4, 4], f32, tag="RA", bufs=2, name="RP")
            if use_cc:
                poutP = dramp.tile([4 * 128, 4], f32, tag="poutA", bufs=2,
                                   name="poutP")
                nc.gpsimd.collective_compute(
                    "AllGather", OP.bypass, replica_groups=rg,
                    ins=[pinP.opt()], outs=[poutP.opt()])
                nc.sync.dma_start(
                    out=RP[:], in_=poutP.rearrange("(r p) f -> p r f", r=4))
            else:
                nc.sync.dma_start(
                    out=RP[:],
                    in_=pinP.rearrange("(r p) f -> p r f", r=1)
                    .to_broadcast((128, 4, 4)))

            # fusion-key attention for return token 2
            kf = []
            for mc in range(4):
                ps = pgen.tile([128, FUS], f32, tag="g", name=f"pkf{mc}")
                for kp in range(2):
                    nc.tensor.matmul(ps[:],
                                     pwkv8[:, kp, :, 128 * mc:128 * (mc + 1)],
                                     tf8[kp][:, :, OWN:TOK],
                                     start=(kp == 0), stop=False,
                                     perf_mode=DR)
                nc.tensor.matmul(ps[:], prow[:, 128 * mc:128 * (mc + 1)],
                                 Sfb[:, OWN:TOK], start=False, stop=True)
                s = ac.tile([128, FUS], bf16, tag=f"kf{mc}", bufs=1,
                            name=f"kfb{mc}")
                nc.vector.tensor_mul(out=s[:], in0=ps[:],
                                     in1=bRf[:, OWN:TOK])
                kf.append(s)
            q2 = []
            for mc in range(4):
                s = ac.tile([128, 32], bf16, tag=f"qf{mc}", bufs=1,
                            name=f"q2b{mc}")
                nc.vector.memset(s[:, 1:32], 0.0)
                nc.vector.tensor_copy(out=s[:, 0:1], in_=pq2s[mc][:])
                q2.append(s)
            e2, l2 = [], []
            for t in range(2):
                sp = pgen.tile([128, FUS], f32, tag="g", name=f"ps2{t}")
                for i in range(4):
                    h = 4 * t + i
                    ch, base = h // 2, (h % 2) * 64
                    nc.tensor.matmul(sp[32 * i:32 * i + 32, :],
                                     q2[ch][base:base + 64, 0:32],
                                     kf[ch][base:base + 64, :],
                                     start=True, stop=True,
                                     tile_position=(base, 32 * i))
                e = ac.tile([128, FUS], bf16, tag=f"e2{t}", bufs=1,
                            name=f"e2{t}")
                la = ac.tile([128, 1], f32, tag=f"la{t}", bufs=2,
                             name=f"pla{t}")
                nc.scalar.activation(out=e[:], in_=sp[:], func=AF.Exp,
                                     scale=1.0 / spl, accum_out=la[:])
                e2.append(e)
                l2.append(la)
            e2T = []
            for t in range(2):
                pt = pgen.tile([FUS, 128], bf16, tag="g", name=f"pet{t}")
                nc.tensor.transpose(pt[:], e2[t][:], ident[:])
                s = ac.tile([FUS, 128], bf16, tag=f"e2T{t}", bufs=1,
                            name=f"e2Tb{t}")
                nc.vector.tensor_scalar_mul(out=s[:], in0=pt[:],
                                            scalar1=rTfs[0:FUS, 3:4])
                e2T.append(s)
            of2 = []
            for t in range(2):
                acc = pacc.tile([128, 64], f32, tag="a", name=f"pacc2{t}")
                for i in range(4):
                    h = 4 * t + i
                    nc.tensor.matmul(acc[32 * i:32 * i + 32, :],
                                     e2T[t][:, 32 * i:32 * i + 32],
                                     Vp[3][0:FUS, DH * h:DH * (h + 1)],
                                     start=True, stop=True,
                                     tile_position=(0, 32 * i))
                li = ac.tile([128, 1], f32, tag="linv", bufs=2,
                             name=f"pli{t}")
                nc.vector.reciprocal(out=li[:], in_=l2[t][:])
                s = ac.tile([128, 64], bf16, tag=f"of{t}", bufs=1,
                            name=f"pof{t}")
                nc.vector.tensor_scalar_mul(out=s[:], in0=acc[:],
                                            scalar1=li[:])
                of2.append(s)
            ofI2 = [ac.tile([128, 1], f8, tag=f"ofI2{kc}", bufs=1,
                            name=f"ofI2{kc}") for kc in range(4)]
            for kc in range(4):
                pt = pacc.tile([128, 32], bf16, tag="a", name=f"ofI2p{kc}")
                for half in range(2):
                    h = 2 * kc + half
                    t, i = h // 4, h % 4
                    nc.tensor.transpose(
                        pt[64 * half:64 * half + 64, :],
                        of2[t][32 * i:32 * i + 32, :],
                        ident[32 * i:32 * i + 32, 32 * i:32 * i + 32],
                        tile_position=(32 * i, 64 * half))
                nc.vector.tensor_copy(out=ofI2[kc][:], in_=pt[:, 0:1])
            P2 = pacc.tile([1, D], f32, tag="a", name="P2")
            for kc in range(4):
                nc.tensor.matmul(P2[:], ofI2[kc][:],
                                 pwo8[:, kc // 2, kc % 2, :],
                                 start=(kc == 0), stop=(kc == 3))
            p2s = ac.tile([1, D], f32, tag="p2s", bufs=1, name="p2s")
            nc.scalar.activation(out=p2s[:], in_=P2[:], func=AF.Copy,
                                 scale=1.0 / spo)
            nc.sync.dma_start(out=out_f, in_=p2s[:])
            # uniform pooled vector u
            pvs = ac.tile([128, 4], f32, tag="vsb", bufs=2, name="pvs")
            nc.vector.tensor_reduce(out=pvs[:],
                                    in_=RP[:].rearrange("p r f -> p f r"),
                                    axis=mybir.AxisListType.X,
                                    op=OP.add)
            nc.vector.tensor_add(out=pvs[:], in0=pvs[:], in1=pvfu[:])
            pvsb8 = ac.tile([128, 4], f8, tag="vsb8", bufs=2,
                            name="pvsb8")
            nc.vector.tensor_copy(out=pvsb8[:], in_=pvs[:])
            pup = pgen.tile([128, 4], f32, tag="g", name="pup")
            for c in range(4):
                for kc in range(4):
                    nc.tensor.matmul(pup[:, c:c + 1],
                                     pwo8[:, kc // 2, kc % 2,
                                          128 * c:128 * (c + 1)],
                                     pvsb8[:, kc:kc + 1],
                                     start=(kc == 0), stop=(kc == 3))
            pus = ac.tile([128, 4], f32, tag="dus", bufs=2, name="pus")
            nc.scalar.activation(out=pus[:], in_=pup[:], func=AF.Copy,
                                 scale=1.0 / (spo * NALL))
            nc.sync.dma_start(out=out_u.rearrange("(c p) one -> p (c one)", c=4),
                              in_=pus[:])


    nc.compile()
    _built[key] = nc
    return nc


def _prep_inputs(inputs):
    """Host-side prep: fold gains, pick fp8 scales, pack weights per layer."""
    I = {k: np.asarray(v, np.float64) for k, v in inputs.items()}
    f32 = np.float32

    def col(x):
        return np.ascontiguousarray(np.asarray(x, f32).reshape(-1, 1))

    scale_dh = DH ** -0.5
    g = I["layers_attn_g"][:, :, None]
    wqf = I["layers_wq"] * scale_dh * g            # [L, 512, 512]
    wkvf = I["layers_wkv"] * g                     # [L, 512, 1024]
    wof = I["layers_wo"]                           # [L, 512, 512]
    w1f = I["layers_ff_w1"] * I["layers_ff_g"][:, :, None]  # [L, 512, 2730]
    w2f = I["layers_ff_w2"]                        # [L, 1365, 512]

    sq = _pow2_scale(wqf)
    skv = _pow2_scale(wkvf)
    so = _pow2_scale(wof)
    s1x = 16.0
    s1g = _pow2_scale(w1f[:, :, IFF:])
    s2 = _pow2_scale(w2f)
    pkvf = I["pool_wkv"] * I["final_g"][:, None]
    spl = _pow2_scale(pkvf)
    spo = _pow2_scale(I["pool_wo"])
    sef = {}
    for mod in ("rna", "atac"):
        ewf = I[f"{mod}_w"] * I[f"{mod}_ln1_g"][:, None]
        sef[mod] = (ewf, _pow2_scale(ewf))
    se = min(sef["rna"][1], sef["atac"][1])
    scales = (sq, skv, so, s1x, s1g, s2, se, spl, spo)

    # packed per-layer fp8 weights
    wpk = np.zeros((DEPTH, 128, FCOLS), F8)
    rows = np.zeros((DEPTH, 3, D), np.float64)
    for l in range(DEPTH):
        wpk[l, :, SEG_WQ:SEG_WKV] = _pack_pairs(wqf[l], sq).reshape(128, -1)
        wpk[l, :, SEG_WKV:SEG_WO] = _pack_pairs(wkvf[l], skv).reshape(128, -1)
        wpk[l, :, SEG_WO:SEG_W1] = _pack_pairs(wof[l], so).reshape(128, -1)
        w1p = np.zeros((D, 2 * IFFP), np.float64)
        w1p[:, 0:IFF] = w1f[l][:, 0:IFF] * s1x
        w1p[:, IFFP:IFFP + IFF] = w1f[l][:, IFF:] * s1g
        wpk[l, :, SEG_W1:SEG_W2] = _pack_pairs(w1p, 1.0).reshape(128, -1)
        w2p = np.zeros((IFF2, D), np.float64)
        w2p[0:IFF, :] = w2f[l] * s2
        wpk[l, :, SEG_W2:FCOLS] = _pack_pairs(w2p, 1.0).reshape(128, -1)
        # rank-1 rows: -scale*colsum/64
        rows[l, 0] = -skv * wkvf[l][:, 0:D].sum(0) / 64     # wksum
        rows[l, 1] = -skv * wkvf[l][:, D:2 * D].sum(0) / 64  # wvsum
        rows[l, 2] = -sq * wqf[l].sum(0) / 64                # wqsum
    rows_b = np.ascontiguousarray(rows.reshape(1, -1)).astype(BF)

    prow = np.zeros((1, 2 * D), np.float64)
    prow[0, 0:D] = -spl * pkvf[:, 0:D].sum(0) / 64
    prow[0, D:2 * D] = -spl * pkvf[:, D:2 * D].sum(0) / 64

    pwkv8 = _pack_pairs(pkvf, spl)
    pwo8 = _pack_pairs(I["pool_wo"], spo)

    # pool query for return token 2 (host, tiny)
    ret = I["return_tokens"].astype(f32)
    gp = I["pool_g"].astype(f32)
    mu = ret.mean(-1, keepdims=True)
    var = ((ret - mu) ** 2).mean(-1, keepdims=True)
    retn = (ret - mu) / np.sqrt(var + 1e-5) * gp
    q2 = (retn[2] @ I["pool_wq"].astype(f32)) * scale_dh

    fus8 = I["fusion_tokens"].T.reshape(4, 128, FUS).transpose(1, 0, 2)

    shared = {
        "wpk": wpk,
        "rows": rows_b,
        "prow": prow.astype(BF),
        "pwkv8": pwkv8,
        "pwo8": pwo8,
        "pool_q2": col(q2),
        "fus_t": np.ascontiguousarray(fus8).astype(np.float32),
    }

    in_maps = []
    for c in range(N_CORES):
        b, q = c // 4, c % 4
        mod = "rna" if q < 2 else "atac"
        x = I[mod][b, (q % 2) * OWN:(q % 2 + 1) * OWN, :]   # [384, 1024]
        m = dict(shared)
        # x8: [128, 4kp, 2sub, 384]
        xT = np.ascontiguousarray(x.T)                      # [1024, 384]
        x8 = _pack_pairs(xT, 1.0)
        m["x8"] = x8
        ewf = sef[mod][0]
        m["ew8"] = _pack_pairs(ewf, se)
        ecols = np.zeros((128, 4, 3), np.float32)
        ecols[:, :, 0] = (I[f"{mod}_b"]
                          + I[f"{mod}_ln1_b"] @ I[f"{mod}_w"]).reshape(4, 128).T
        ecols[:, :, 1] = I[f"{mod}_ln2_g"].reshape(4, 128).T
        ecols[:, :, 2] = I[f"{mod}_ln2_b"].reshape(4, 128).T
        m["ecols"] = ecols
        # embed input-LN stats from the quantized x8 the device would see
        xq = x8.astype(np.float64).transpose(1, 2, 0, 3).reshape(RIN, OWN)
        Sr = xq.sum(0) / 8.0
        mu = Sr / 128.0
        var = (xq * xq).mean(0) - mu * mu
        rstd = 1.0 / np.sqrt(var + 1e-5)
        erows = np.zeros(D + 2 * OWN, np.float64)
        erows[0:D] = -se * ewf.sum(0) / 128
        erows[D:D + OWN] = Sr
        erows[D + OWN:] = rstd / se
        m["erows"] = erows.reshape(1, -1).astype(BF)
        in_maps.append(m)
    return in_maps, ret.astype(f32), scales


def kernel(**inputs):
    from concourse import bass_utils
    in_maps, ret, scales = _prep_inputs(inputs)
    nc = build(num_devices=N_CORES, use_cc=True, scales=scales)
    res = bass_utils.run_bass_kernel_spmd(nc, in_maps,
                                          core_ids=list(range(N_CORES)))
    out = np.zeros((B, 3, D), np.float32)
    for b in range(2):
        r = res.results[4 * b]
        u = r["out_u"][:, 0]
        f = r["out_f"][0]
        out[b, 0] = u + ret[0]
        out[b, 1] = u + ret[1]
        out[b, 2] = f + ret[2]
    return out
